# revision 1
# baseline (speedup 1.0000x reference)
"""ConvAttention TRN2 Bass kernel.

Sharding: 16 (batch, head) pairs over 8 cores -> each core handles one batch b
and a head-pair (heads 0,1 or 2,3). Each core computes a partial output
y_part = w_out[:, pair_slice] @ attn_out_pair  [256, 4096] fp32; host sums the
two partials per batch and adds b_out.

Per-core pipeline (all SPMD-identical, different data):
  phase0: qkv projections (bf16 matmuls), q/k replicated x3 across PE row
          quadrants for tile_position packing; v transposed via PE into
          v_ext (ones column appended -> softmax denominator for free).
  phase1: per (head, i-chunk of 512): sim_T[j,i] = k^T q on PE (3-way row
          packing, K=32), exp on ScalarE (PSUM->SBUF bf16, SCALE folded),
          out_T[d,i] = v_ext^T p_T accumulated over j-tiles with 2-way
          column packing (even/odd j-tiles to col quadrants 0/64).
  phase2: per (head, i-half): PE-transpose out_ext (A+B accumulated in
          PSUM), reciprocal of denominator, per-partition broadcast mult,
          PE-transpose back -> out_norm [64, n] bf16.
  phase3: y = w_outT^T @ out_norm -> DRAM fp32.
"""

import numpy as np
import ml_dtypes

import concourse.bass as bass
import concourse.bacc as bacc
import concourse.mybir as mybir
import concourse.tile as tile
from concourse import bass_utils
from concourse.masks import make_identity

from concourse.dve_spec import (
    Spec, Src0, C0, C1, C2, One, sq,
    lower as _dve_lower, _has_src1,
)
import concourse.dve_ops as _dops
from concourse.dve_uop import DveOpSpec as _DveOpSpec

BF16 = mybir.dt.bfloat16
F32 = mybir.dt.float32
AF = mybir.ActivationFunctionType


def _exp8_ref(in0, in1, c0, c1, c2):
    x = np.asarray(in0, np.float32)
    t = (x * np.float32(c0)).astype(np.float32)
    y0 = ((np.float32(1.0) + t) + (t * t) * np.float32(c1)).astype(np.float32)
    y = (y0 * y0).astype(np.float32)
    y = (y * y).astype(np.float32)
    return (y * y).astype(np.float32)


def _register_exp8():
    # exp(s0*8*x) ~= ((1+t) + t^2*(1/2 + t/6))^8, t = s0*x.  8 ALU stages.
    name = "EXP8_ANT"
    for op in _dops.OPS:
        if op.name == name:
            return op
    t = Src0 * C0
    body = sq(sq(sq((One + t) + sq(t) * C1)))
    spec = Spec(body=body, reference=_exp8_ref)
    row = max(_dops._SUB_OPCODE_FOR_NAME.values()) + 1
    _dops._SUB_OPCODE_FOR_NAME[name] = row
    shas = {}
    for ver in ("v3", "v4"):
        try:
            uops = _dve_lower(spec, ver=ver)
            shas[ver] = _DveOpSpec(name=name, opcode=row, uops=uops,
                                   rd1_en=_has_src1(spec)).sha(ver)
        except Exception:
            pass
    op = _dops.DveOp(name, spec, subdim=False, uops_sha=shas)
    _dops.OPS.append(op)
    _dops.CUSTOM_DVE_SPECS[name] = spec
    return op


EXP8 = _register_exp8()
# softmax-exp groups routed to the Vector engine (rest go to ScalarE)
DVE_GROUPS = frozenset({1, 3, 5, 8, 10, 12, 14})

HEADS = 4
DIM_HEAD = 32
SCALE = DIM_HEAD ** (-0.5)
B, C, H, W = 4, 256, 64, 64
N = H * W            # 4096
NT = N // 128        # 32 j-tiles
IC = 512             # i-chunk
NIC = N // IC        # 8 i-chunks
NG = NT // 2  # 16 groups of 2 j-tiles (2-way PE row packing)


def build_program(nc, tc, phases=3):
    """Emit the per-core program. DRAM tensor names are the in_map keys."""
    xb = nc.dram_tensor("xb", [2, 128, N], BF16, kind="ExternalInput").ap()
    wq0 = nc.dram_tensor("wq0", [128, 128], BF16, kind="ExternalInput").ap()
    wq1 = nc.dram_tensor("wq1", [128, 128], BF16, kind="ExternalInput").ap()
    wk0 = nc.dram_tensor("wk0", [128, 128], BF16, kind="ExternalInput").ap()
    wk1 = nc.dram_tensor("wk1", [128, 128], BF16, kind="ExternalInput").ap()
    wv = nc.dram_tensor("wv", [128, 194], BF16, kind="ExternalInput").ap()
    wo = nc.dram_tensor("wo", [64, 256], BF16, kind="ExternalInput").ap()
    y = nc.dram_tensor("y", [256, N], F32, kind="ExternalOutput").ap()

    with (
        tc.tile_pool(name="singles", bufs=1) as singles,
        tc.tile_pool(name="ppool", bufs=16) as ppool,
        tc.tile_pool(name="opool", bufs=3) as opool,
        tc.tile_pool(name="mpool", bufs=2) as mpool,
        tc.tile_pool(name="ypool", bufs=2) as ypool,
        tc.tile_pool(name="psum", bufs=2, space="PSUM") as psum,
    ):
        ident_f = singles.tile([128, 128], F32)
        ident_b = singles.tile([128, 128], BF16)
        make_identity(nc, ident_f[:])
        make_identity(nc, ident_b[:])

        sb_wq = [singles.tile([128, 128], BF16, tag=f"wq{j}", name=f"sb_wq{j}") for j in range(2)]
        sb_wk = [singles.tile([128, 128], BF16, tag=f"wk{j}", name=f"sb_wk{j}") for j in range(2)]
        sb_wv = singles.tile([128, 194], BF16)
        sb_wo = singles.tile([64, 256], BF16)
        nc.sync.dma_start(out=sb_wq[0][:], in_=wq0)
        nc.sync.dma_start(out=sb_wq[1][:], in_=wq1)
        nc.sync.dma_start(out=sb_wk[0][:], in_=wk0)
        nc.sync.dma_start(out=sb_wk[1][:], in_=wk1)
        nc.sync.dma_start(out=sb_wv[:], in_=wv)
        nc.sync.dma_start(out=sb_wo[:], in_=wo)

        sb_x = [singles.tile([128, N], BF16, tag=f"x{cc}", name=f"sb_x{cc}") for cc in range(2)]
        nc.sync.dma_start(out=sb_x[0][:], in_=xb[0])
        nc.sync.dma_start(out=sb_x[1][:], in_=xb[1])

        # ---- phase 0: projections --------------------------------------
        q_rep = [singles.tile([64, N], BF16, tag=f"qr{j}", name=f"q_rep{j}") for j in range(2)]
        k_rep = [singles.tile([64, N], BF16, tag=f"kr{j}", name=f"k_rep{j}") for j in range(2)]
        v2 = singles.tile([97, N], BF16)

        NCH = [(i * 1024, 1024) for i in range(4)]
        projs = [
            (sb_wq[0], 64, q_rep[0][:]), (sb_wq[1], 64, q_rep[1][:]),
            (sb_wk[0], 64, k_rep[0][:]), (sb_wk[1], 64, k_rep[1][:]),
            (sb_wv, 97, v2[:]),
        ]
        for w_sb, m, dst in projs:
            for n0, nw in NCH:
                ps = psum.tile([128, 1024], F32, tag="sim")
                for s in range(nw // 512):
                    for cc in range(2):
                        nc.tensor.matmul(
                            ps[0:m, s * 512:(s + 1) * 512],
                            lhsT=w_sb[:, cc * m:(cc + 1) * m],
                            rhs=sb_x[cc][:, n0 + s * 512:n0 + (s + 1) * 512],
                            start=(cc == 0), stop=(cc == 1),
                        )
                nc.any.tensor_copy(dst[0:m, n0:n0 + nw], ps[0:m, 0:nw])
        # ones rows for the denominator column of v_ext
        nc.vector.memset(v2[32:33, :], 1.0)
        nc.vector.memset(v2[96:97, :], 1.0)

        # v_ext_all[:, jt*66 + 33h : +33] = [v_h^T | ones] for j-tile jt
        v_ext = singles.tile([128, NT * 98], BF16)
        for b8 in range(NT // 8):
            vt = psum.tile([128, 8 * 98], BF16, tag="sim")
            for s in range(8):
                jt = b8 * 8 + s
                nc.tensor.matmul(
                    vt[:, s * 98:s * 98 + 97],
                    lhsT=v2[0:97, jt * 128:(jt + 1) * 128],
                    rhs=ident_b[0:97, 0:97],
                    is_transpose=True,
                )
            nc.vector.tensor_copy(
                v_ext[:, b8 * 8 * 98:(b8 + 1) * 8 * 98]
                    .rearrange("p (s c) -> p s c", c=98)[:, :, 0:97],
                vt[:].rearrange("p (s c) -> p s c", c=98)[:, :, 0:97])

        # ---- phases 1-3 ------------------------------------------------
        on_sb = singles.tile([64, N], BF16)  # normalized attn out, both heads

        if phases == 0:
            dbg = singles.tile([128, N], F32, name="dbg")
            nc.any.tensor_copy(dbg[0:64, 0:N], q_rep[0][0:64, :])
            nc.any.tensor_copy(dbg[64:128, 0:N // 2],
                               v_ext[0:64, 0:N // 2])
            nc.sync.dma_start(out=y[0:128, :], in_=dbg[:])
            return

        for half in range(2):
            for h in range(2):
                oe = opool.tile([97, N // 2], F32, tag="oext")
                nc.vector.memset(oe[32:64, :], 0.0)
                for icl in range(NIC // 2):
                    ic0 = half * (N // 2) + icl * IC
                    # sim + exp for all 32 j-tiles at this i-chunk
                    p3s = []
                    for g in range(NG):
                        sp = psum.tile([128, 1024], F32, tag="sim")
                        for q in range(2):
                            jt = 2 * g + q
                            nc.tensor.matmul(
                                sp[:, q * 512:(q + 1) * 512],
                                lhsT=k_rep[h][32 * q:32 * q + 32,
                                              jt * 128:(jt + 1) * 128],
                                rhs=q_rep[h][32 * q:32 * q + 32, ic0:ic0 + IC],
                                start=True, stop=True,
                                tile_position=(32 * q, 0),
                            )
                        p3 = ppool.tile([128, 1024], BF16, tag="p3")
                        if g in DVE_GROUPS:
                            nc.vector._custom_dve(
                                EXP8, out=p3[:], in0=sp[:],
                                s0=SCALE / 8.0, s1=0.5, imm2=0.0)
                        else:
                            nc.scalar.activation(p3[:], sp[:], AF.Exp,
                                                 scale=SCALE)
                        p3s.append(p3)
                    # out matmul: accumulate over j-tiles; even j-tiles go to
                    # bank 0 rows 0-32, odd to bank 1 rows 64-96 (col packing)
                    op = psum.tile([97, 2 * IC], F32, tag="out", bufs=1)
                    for jt in range(NT):
                        g, q = jt // 2, jt % 2
                        r0 = 64 * q
                        nc.tensor.matmul(
                            op[r0:r0 + 33, q * IC:(q + 1) * IC],
                            lhsT=v_ext[:, jt * 98 + 64 * h:jt * 98 + 64 * h + 33],
                            rhs=p3s[g][:, q * 512:(q + 1) * 512],
                            start=(jt < 2), stop=(jt >= NT - 2),
                            tile_position=(0, r0),
                        )
                    icl0 = icl * IC
                    nc.vector.tensor_copy(oe[0:33, icl0:icl0 + IC],
                                          op[0:33, 0:IC])
                    nc.vector.tensor_copy(oe[64:97, icl0:icl0 + IC],
                                          op[64:97, IC:2 * IC])

                if phases == 1:
                    if half == 0 and h == 0:
                        nc.sync.dma_start(out=y[0:97, 0:N // 2], in_=oe[:])
                    continue

                # phase 2: transpose, normalize, transpose back
                outT = mpool.tile([128, 16 * 33], F32, tag="outT")
                for b4 in range(4):
                    tp = psum.tile([128, 4 * 98], F32, tag="small")
                    for s in range(4):
                        it = b4 * 4 + s
                        nc.tensor.matmul(
                            tp[:, s * 98:s * 98 + 97],
                            lhsT=oe[0:97, it * 128:(it + 1) * 128],
                            rhs=ident_f[0:97, 0:97],
                            is_transpose=True,
                        )
                    dst = outT[:, b4 * 132:(b4 + 1) * 132] \
                        .rearrange("p (s c) -> p s c", c=33)
                    tpv = tp[:].rearrange("p (s c) -> p s c", c=98)
                    nc.vector.tensor_copy(dst, tpv[:, :, 0:33])
                    nc.vector.tensor_add(dst, dst, tpv[:, :, 64:97])
                if phases in (2.05, 2.1):
                    continue
                outT_v = outT[:].rearrange("p (t c) -> p t c", c=33)
                recip = mpool.tile([128, 16], F32, tag="recip")
                nc.vector.reciprocal(recip[:], outT_v[:, :, 32:33])
                onT = mpool.tile([128, 512], BF16, tag="onT")
                for t in range(16):
                    nc.vector.tensor_scalar_mul(
                        onT[:, t * 32:(t + 1) * 32],
                        outT_v[:, t, 0:32],
                        recip[:, t:t + 1],
                    )
                if phases == 2.2:
                    continue
                for b4 in range(4):
                    tb = psum.tile([64, 512], BF16, tag="small")
                    for s in range(4):
                        it = b4 * 4 + s
                        nc.tensor.matmul(
                            tb[32 * h:32 * h + 32, s * 128:(s + 1) * 128],
                            lhsT=onT[:, it * 32:(it + 1) * 32],
                            rhs=ident_b[:, 0:128],
                            is_transpose=True,
                            tile_position=(0, 32 * h),
                        )
                    dst0 = half * (N // 2) + b4 * 512
                    nc.vector.tensor_copy(
                        on_sb[32 * h:32 * h + 32, dst0:dst0 + 512],
                        tb[32 * h:32 * h + 32, :],
                    )

            if phases < 3:
                continue
            # phase 3: output projection for this i-half
            for ot in range(2):
                ysb = ypool.tile([128, N // 2], F32, tag="ysb")
                for icc in range(NIC // 2):
                    yp = psum.tile([128, IC], F32, tag="small")
                    s0 = half * (N // 2) + icc * IC
                    nc.tensor.matmul(
                        yp[:],
                        lhsT=sb_wo[:, ot * 128:(ot + 1) * 128],
                        rhs=on_sb[0:64, s0:s0 + IC],
                        start=True, stop=True,
                    )
                    nc.any.tensor_copy(ysb[:, icc * IC:(icc + 1) * IC], yp[:])
                nc.sync.dma_start(
                    out=y[ot * 128:(ot + 1) * 128,
                          half * (N // 2):(half + 1) * (N // 2)],
                    in_=ysb[:],
                )


_CACHE = {}


def get_compiled(phases=3):
    key = ("nc", phases)
    if key not in _CACHE:
        nc = bacc.Bacc("TRN2", target_bir_lowering=False, debug=False,
                       num_devices=8)
        with tile.TileContext(nc) as tc:
            build_program(nc, tc, phases=phases)
        nc.compile()
        _CACHE[key] = nc
    return _CACHE[key]


def _bf(a):
    return np.ascontiguousarray(a.astype(ml_dtypes.bfloat16))


def prep_core_inputs(x, w_qkv, w_out):
    """Build the 8 per-core in_maps (host-side slicing/transposes)."""
    x = np.asarray(x, np.float32).reshape(B, C, N)
    w_qkv = np.asarray(w_qkv, np.float32)
    w_out = np.asarray(w_out, np.float32)
    in_maps = []
    for core in range(8):
        b, pair = core // 2, core % 2
        ha, hb = 2 * pair, 2 * pair + 1
        xb = _bf(x[b].reshape(2, 128, N))

        def rep2(rows):  # [32, 256] weight rows -> [128, 128] replicated x2
            out = np.zeros((128, 128), np.float32)
            for cc in range(2):
                blk = rows[:, cc * 128:(cc + 1) * 128].T  # [128c, 32d]
                for r in range(2):
                    out[:, cc * 64 + r * 32: cc * 64 + (r + 1) * 32] = blk
            return _bf(out)

        wq = [rep2(w_qkv[32 * hh:32 * hh + 32]) for hh in (ha, hb)]
        wk = [rep2(w_qkv[128 + 32 * hh:128 + 32 * hh + 32]) for hh in (ha, hb)]
        wv = np.zeros((128, 194), np.float32)
        for cc in range(2):
            wv[:, cc * 97 + 0: cc * 97 + 32] = \
                w_qkv[256 + 32 * ha:256 + 32 * ha + 32, cc * 128:(cc + 1) * 128].T
            wv[:, cc * 97 + 64: cc * 97 + 96] = \
                w_qkv[256 + 32 * hb:256 + 32 * hb + 32, cc * 128:(cc + 1) * 128].T
        wo = np.concatenate(
            [w_out[:, 32 * ha:32 * ha + 32].T, w_out[:, 32 * hb:32 * hb + 32].T],
            axis=0)  # [64, 256]
        in_maps.append({
            "xb": xb, "wq0": wq[0], "wq1": wq[1], "wk0": wk[0], "wk1": wk[1],
            "wv": _bf(wv), "wo": _bf(wo),
        })
    return in_maps


def run_cores(in_maps, trace=False):
    nc = get_compiled()
    res = bass_utils.run_bass_kernel_spmd(
        nc, in_maps, core_ids=list(range(8)), trace=trace)
    return res


def assemble_output(results, b_out):
    b_out = np.asarray(b_out, np.float32)
    y = np.zeros((B, C, N), np.float32)
    for b in range(B):
        y[b] = results[2 * b]["y"] + results[2 * b + 1]["y"]
    y += b_out[None, :, None]
    return y.reshape(B, C, H, W)


def kernel(x, w_qkv, w_out, b_out):
    in_maps = prep_core_inputs(x, w_qkv, w_out)
    res = run_cores(in_maps)
    return assemble_output(res.results, b_out)



# revision 11
# speedup vs baseline: 1.7922x; 1.7922x over previous
"""ConvAttention TRN2 Bass kernel.

Sharding: 16 (batch, head) pairs over 8 cores -> each core handles one batch b
and a head-pair (heads 0,1 or 2,3). Each core computes a partial output
y_part = w_out[:, pair_slice] @ attn_out_pair  [256, 4096] bf16.

Host I/O over the axon tunnel dominates wall time (~40 MiB/s), so the wire
format is minimized: each core uploads only HALF of its batch's x (1 MiB bf16;
the pair AllGathers the full x on device), and downloads only its 128-channel
shard of the final y (1 MiB bf16) after an on-device ReduceScatter(add) over
the pair's partials. b_out is added on host.

Per-core pipeline (all SPMD-identical, different data):
  phase0: qkv projections (bf16 matmuls), q/k replicated x3 across PE row
          quadrants for tile_position packing; v transposed via PE into
          v_ext (ones column appended -> softmax denominator for free).
  phase1: per (head, i-chunk of 512): sim_T[j,i] = k^T q on PE (3-way row
          packing, K=32), exp on ScalarE (PSUM->SBUF bf16, SCALE folded),
          out_T[d,i] = v_ext^T p_T accumulated over j-tiles with 2-way
          column packing (even/odd j-tiles to col quadrants 0/64).
  phase2: per (head, i-half): PE-transpose out_ext (A+B accumulated in
          PSUM), reciprocal of denominator, per-partition broadcast mult,
          PE-transpose back -> out_norm [64, n] bf16.
  phase3: y = w_outT^T @ out_norm -> DRAM fp32.
"""

import numpy as np
import ml_dtypes

import concourse.bass as bass
import concourse.bacc as bacc
import concourse.mybir as mybir
import concourse.tile as tile
from concourse import bass_utils
from concourse.masks import make_identity

from concourse.dve_spec import (
    Spec, Src0, C0, C1, C2, One, sq,
    lower as _dve_lower, _has_src1,
)
import concourse.dve_ops as _dops
from concourse.dve_uop import DveOpSpec as _DveOpSpec

BF16 = mybir.dt.bfloat16
F32 = mybir.dt.float32
AF = mybir.ActivationFunctionType


def _exp8_ref(in0, in1, c0, c1, c2):
    x = np.asarray(in0, np.float32)
    t = (x * np.float32(c0)).astype(np.float32)
    y0 = ((np.float32(1.0) + t) + (t * t) * np.float32(c1)).astype(np.float32)
    y = (y0 * y0).astype(np.float32)
    y = (y * y).astype(np.float32)
    return (y * y).astype(np.float32)


def _register_exp8():
    # exp(s0*8*x) ~= ((1+t) + t^2*(1/2 + t/6))^8, t = s0*x.  8 ALU stages.
    name = "EXP8_ANT"
    for op in _dops.OPS:
        if op.name == name:
            return op
    t = Src0 * C0
    body = sq(sq(sq((One + t) + sq(t) * C1)))
    spec = Spec(body=body, reference=_exp8_ref)
    row = max(_dops._SUB_OPCODE_FOR_NAME.values()) + 1
    _dops._SUB_OPCODE_FOR_NAME[name] = row
    shas = {}
    for ver in ("v3", "v4"):
        try:
            uops = _dve_lower(spec, ver=ver)
            shas[ver] = _DveOpSpec(name=name, opcode=row, uops=uops,
                                   rd1_en=_has_src1(spec)).sha(ver)
        except Exception:
            pass
    op = _dops.DveOp(name, spec, subdim=False, uops_sha=shas)
    _dops.OPS.append(op)
    _dops.CUSTOM_DVE_SPECS[name] = spec
    return op


EXP8 = _register_exp8()
# softmax-exp groups routed to the Vector engine (rest go to ScalarE)
DVE_GROUPS = frozenset({1, 3, 5, 8, 10, 12, 14})

HEADS = 4
DIM_HEAD = 32
SCALE = DIM_HEAD ** (-0.5)
B, C, H, W = 4, 256, 64, 64
N = H * W            # 4096
NT = N // 128        # 32 j-tiles
IC = 512             # i-chunk
NIC = N // IC        # 8 i-chunks
NG = NT // 2  # 16 groups of 2 j-tiles (2-way PE row packing)


PAIRS = [[0, 1], [2, 3], [4, 5], [6, 7]]


def build_program(nc, tc, phases=3):
    """Emit the per-core program. DRAM tensor names are the in_map keys."""
    xh = nc.dram_tensor("xh", [256, N // 2], BF16, kind="ExternalInput").ap()
    wq0 = nc.dram_tensor("wq0", [128, 128], BF16, kind="ExternalInput").ap()
    wq1 = nc.dram_tensor("wq1", [128, 128], BF16, kind="ExternalInput").ap()
    wk0 = nc.dram_tensor("wk0", [128, 128], BF16, kind="ExternalInput").ap()
    wk1 = nc.dram_tensor("wk1", [128, 128], BF16, kind="ExternalInput").ap()
    wv = nc.dram_tensor("wv", [128, 194], BF16, kind="ExternalInput").ap()
    wo = nc.dram_tensor("wo", [64, 256], BF16, kind="ExternalInput").ap()
    yh = nc.dram_tensor("yh", [128, N], BF16, kind="ExternalOutput").ap()

    with (
        tc.tile_pool(name="singles", bufs=1) as singles,
        tc.tile_pool(name="ppool", bufs=16) as ppool,
        tc.tile_pool(name="opool", bufs=3) as opool,
        tc.tile_pool(name="mpool", bufs=2) as mpool,
        tc.tile_pool(name="ypool", bufs=2) as ypool,
        tc.tile_pool(name="dram", bufs=1, space="DRAM") as dram,
        tc.tile_pool(name="psum", bufs=2, space="PSUM") as psum,
    ):
        # AllGather x within batch pairs: each core uploads half of its
        # batch's x; xg = [half0 rows ; half1 rows], identical on the pair.
        xh_b = dram.tile([256, N // 2], BF16)
        xg = dram.tile([512, N // 2], BF16)
        nc.gpsimd.dma_start(out=xh_b[:], in_=xh)
        nc.gpsimd.collective_compute(
            "AllGather", mybir.AluOpType.bypass, replica_groups=PAIRS,
            ins=[xh_b[:].opt()], outs=[xg[:].opt()],
        )
        ypart = dram.tile([256, N], BF16)
        yred = dram.tile([128, N], BF16)
        ident_f = singles.tile([128, 128], F32)
        ident_b = singles.tile([128, 128], BF16)
        make_identity(nc, ident_f[:])
        make_identity(nc, ident_b[:])

        sb_wq = [singles.tile([128, 128], BF16, tag=f"wq{j}", name=f"sb_wq{j}") for j in range(2)]
        sb_wk = [singles.tile([128, 128], BF16, tag=f"wk{j}", name=f"sb_wk{j}") for j in range(2)]
        sb_wv = singles.tile([128, 194], BF16)
        sb_wo = singles.tile([64, 256], BF16)
        nc.sync.dma_start(out=sb_wq[0][:], in_=wq0)
        nc.sync.dma_start(out=sb_wq[1][:], in_=wq1)
        nc.sync.dma_start(out=sb_wk[0][:], in_=wk0)
        nc.sync.dma_start(out=sb_wk[1][:], in_=wk1)
        nc.sync.dma_start(out=sb_wv[:], in_=wv)
        nc.sync.dma_start(out=sb_wo[:], in_=wo)

        sb_x = [singles.tile([128, N], BF16, tag=f"x{cc}", name=f"sb_x{cc}") for cc in range(2)]
        for cc in range(2):
            for half in range(2):
                nc.sync.dma_start(
                    out=sb_x[cc][:, half * (N // 2):(half + 1) * (N // 2)],
                    in_=xg[half * 256 + cc * 128: half * 256 + (cc + 1) * 128, :],
                )

        # ---- phase 0: projections --------------------------------------
        q_rep = [singles.tile([64, N], BF16, tag=f"qr{j}", name=f"q_rep{j}") for j in range(2)]
        k_rep = [singles.tile([64, N], BF16, tag=f"kr{j}", name=f"k_rep{j}") for j in range(2)]
        v2 = singles.tile([97, N], BF16)

        NCH = [(i * 1024, 1024) for i in range(4)]
        projs = [
            (sb_wq[0], 64, q_rep[0][:]), (sb_wq[1], 64, q_rep[1][:]),
            (sb_wk[0], 64, k_rep[0][:]), (sb_wk[1], 64, k_rep[1][:]),
            (sb_wv, 97, v2[:]),
        ]
        for w_sb, m, dst in projs:
            for n0, nw in NCH:
                ps = psum.tile([128, 1024], F32, tag="sim")
                for s in range(nw // 512):
                    for cc in range(2):
                        nc.tensor.matmul(
                            ps[0:m, s * 512:(s + 1) * 512],
                            lhsT=w_sb[:, cc * m:(cc + 1) * m],
                            rhs=sb_x[cc][:, n0 + s * 512:n0 + (s + 1) * 512],
                            start=(cc == 0), stop=(cc == 1),
                        )
                nc.any.tensor_copy(dst[0:m, n0:n0 + nw], ps[0:m, 0:nw])
        # ones rows for the denominator column of v_ext
        nc.vector.memset(v2[32:33, :], 1.0)
        nc.vector.memset(v2[96:97, :], 1.0)

        # v_ext_all[:, jt*66 + 33h : +33] = [v_h^T | ones] for j-tile jt
        v_ext = singles.tile([128, NT * 98], BF16)
        for b8 in range(NT // 8):
            vt = psum.tile([128, 8 * 98], BF16, tag="sim")
            for s in range(8):
                jt = b8 * 8 + s
                nc.tensor.matmul(
                    vt[:, s * 98:s * 98 + 97],
                    lhsT=v2[0:97, jt * 128:(jt + 1) * 128],
                    rhs=ident_b[0:97, 0:97],
                    is_transpose=True,
                )
            nc.vector.tensor_copy(
                v_ext[:, b8 * 8 * 98:(b8 + 1) * 8 * 98]
                    .rearrange("p (s c) -> p s c", c=98)[:, :, 0:97],
                vt[:].rearrange("p (s c) -> p s c", c=98)[:, :, 0:97])

        # ---- phases 1-3 ------------------------------------------------
        on_sb = singles.tile([64, N], BF16)  # normalized attn out, both heads

        if phases == 0:
            dbg = singles.tile([128, N], BF16, name="dbg")
            nc.any.tensor_copy(dbg[0:64, 0:N], q_rep[0][0:64, :])
            nc.any.tensor_copy(dbg[64:128, 0:N // 2],
                               v_ext[0:64, 0:N // 2])
            nc.sync.dma_start(out=yh, in_=dbg[:])
            return

        for half in range(2):
            for h in range(2):
                oe = opool.tile([97, N // 2], F32, tag="oext")
                nc.vector.memset(oe[32:64, :], 0.0)
                for icl in range(NIC // 2):
                    ic0 = half * (N // 2) + icl * IC
                    # sim + exp for all 32 j-tiles at this i-chunk
                    p3s = []
                    for g in range(NG):
                        sp = psum.tile([128, 1024], F32, tag="sim")
                        for q in range(2):
                            jt = 2 * g + q
                            nc.tensor.matmul(
                                sp[:, q * 512:(q + 1) * 512],
                                lhsT=k_rep[h][32 * q:32 * q + 32,
                                              jt * 128:(jt + 1) * 128],
                                rhs=q_rep[h][32 * q:32 * q + 32, ic0:ic0 + IC],
                                start=True, stop=True,
                                tile_position=(32 * q, 0),
                            )
                        p3 = ppool.tile([128, 1024], BF16, tag="p3")
                        if g in DVE_GROUPS:
                            nc.vector._custom_dve(
                                EXP8, out=p3[:], in0=sp[:],
                                s0=SCALE / 8.0, s1=0.5, imm2=0.0)
                        else:
                            nc.scalar.activation(p3[:], sp[:], AF.Exp,
                                                 scale=SCALE)
                        p3s.append(p3)
                    # out matmul: accumulate over j-tiles; even j-tiles go to
                    # bank 0 rows 0-32, odd to bank 1 rows 64-96 (col packing)
                    op = psum.tile([97, 2 * IC], F32, tag="out", bufs=1)
                    for jt in range(NT):
                        g, q = jt // 2, jt % 2
                        r0 = 64 * q
                        nc.tensor.matmul(
                            op[r0:r0 + 33, q * IC:(q + 1) * IC],
                            lhsT=v_ext[:, jt * 98 + 64 * h:jt * 98 + 64 * h + 33],
                            rhs=p3s[g][:, q * 512:(q + 1) * 512],
                            start=(jt < 2), stop=(jt >= NT - 2),
                            tile_position=(0, r0),
                        )
                    icl0 = icl * IC
                    nc.vector.tensor_copy(oe[0:33, icl0:icl0 + IC],
                                          op[0:33, 0:IC])
                    nc.vector.tensor_copy(oe[64:97, icl0:icl0 + IC],
                                          op[64:97, IC:2 * IC])

                if phases == 1:
                    if half == 0 and h == 0:
                        dbg1 = singles.tile([97, N // 2], BF16, name="dbg1")
                        nc.vector.tensor_copy(dbg1[:], oe[:])
                        nc.sync.dma_start(out=yh[0:97, 0:N // 2], in_=dbg1[:])
                    continue

                # phase 2: transpose, normalize, transpose back
                outT = mpool.tile([128, 16 * 33], F32, tag="outT")
                for b4 in range(4):
                    tp = psum.tile([128, 4 * 98], F32, tag="small")
                    for s in range(4):
                        it = b4 * 4 + s
                        nc.tensor.matmul(
                            tp[:, s * 98:s * 98 + 97],
                            lhsT=oe[0:97, it * 128:(it + 1) * 128],
                            rhs=ident_f[0:97, 0:97],
                            is_transpose=True,
                        )
                    dst = outT[:, b4 * 132:(b4 + 1) * 132] \
                        .rearrange("p (s c) -> p s c", c=33)
                    tpv = tp[:].rearrange("p (s c) -> p s c", c=98)
                    nc.vector.tensor_copy(dst, tpv[:, :, 0:33])
                    nc.vector.tensor_add(dst, dst, tpv[:, :, 64:97])
                if phases in (2.05, 2.1):
                    continue
                outT_v = outT[:].rearrange("p (t c) -> p t c", c=33)
                recip = mpool.tile([128, 16], F32, tag="recip")
                nc.vector.reciprocal(recip[:], outT_v[:, :, 32:33])
                onT = mpool.tile([128, 512], BF16, tag="onT")
                for t in range(16):
                    nc.vector.tensor_scalar_mul(
                        onT[:, t * 32:(t + 1) * 32],
                        outT_v[:, t, 0:32],
                        recip[:, t:t + 1],
                    )
                if phases == 2.2:
                    continue
                for b4 in range(4):
                    tb = psum.tile([64, 512], BF16, tag="small")
                    for s in range(4):
                        it = b4 * 4 + s
                        nc.tensor.matmul(
                            tb[32 * h:32 * h + 32, s * 128:(s + 1) * 128],
                            lhsT=onT[:, it * 32:(it + 1) * 32],
                            rhs=ident_b[:, 0:128],
                            is_transpose=True,
                            tile_position=(0, 32 * h),
                        )
                    dst0 = half * (N // 2) + b4 * 512
                    nc.vector.tensor_copy(
                        on_sb[32 * h:32 * h + 32, dst0:dst0 + 512],
                        tb[32 * h:32 * h + 32, :],
                    )

            if phases < 3:
                continue
            # phase 3: output projection for this i-half
            for ot in range(2):
                ysb = ypool.tile([128, N // 2], BF16, tag="ysb")
                for icc in range(NIC // 2):
                    yp = psum.tile([128, IC], F32, tag="small")
                    s0 = half * (N // 2) + icc * IC
                    nc.tensor.matmul(
                        yp[:],
                        lhsT=sb_wo[:, ot * 128:(ot + 1) * 128],
                        rhs=on_sb[0:64, s0:s0 + IC],
                        start=True, stop=True,
                    )
                    nc.any.tensor_copy(ysb[:, icc * IC:(icc + 1) * IC], yp[:])
                nc.sync.dma_start(
                    out=ypart[ot * 128:(ot + 1) * 128,
                              half * (N // 2):(half + 1) * (N // 2)],
                    in_=ysb[:],
                )

        if phases >= 3:
            # sum the two per-pair partials on device; even core keeps
            # channels 0:128, odd core channels 128:256.
            nc.gpsimd.collective_compute(
                "ReduceScatter", mybir.AluOpType.add, replica_groups=PAIRS,
                ins=[ypart[:].opt()], outs=[yred[:].opt()],
            )
            nc.sync.dma_start(out=yh, in_=yred[:])


_CACHE = {}


def get_compiled(phases=3):
    key = ("nc", phases)
    if key not in _CACHE:
        nc = bacc.Bacc("TRN2", target_bir_lowering=False, debug=False,
                       num_devices=8)
        with tile.TileContext(nc) as tc:
            build_program(nc, tc, phases=phases)
        nc.compile()
        _CACHE[key] = nc
    return _CACHE[key]


def _bf(a):
    return np.ascontiguousarray(a.astype(ml_dtypes.bfloat16))


def prep_core_inputs(x, w_qkv, w_out):
    """Build the 8 per-core in_maps (host-side slicing/transposes)."""
    x = np.asarray(x, np.float32).reshape(B, C, N)
    w_qkv = np.asarray(w_qkv, np.float32)
    w_out = np.asarray(w_out, np.float32)
    in_maps = []
    for core in range(8):
        b, pair = core // 2, core % 2
        ha, hb = 2 * pair, 2 * pair + 1
        # this core uploads spatial half `pair` of batch b; the pair
        # AllGathers to reconstruct the full x on device.
        xh = _bf(x[b].reshape(2 * 128, N)[:, pair * (N // 2):(pair + 1) * (N // 2)])

        def rep2(rows):  # [32, 256] weight rows -> [128, 128] replicated x2
            out = np.zeros((128, 128), np.float32)
            for cc in range(2):
                blk = rows[:, cc * 128:(cc + 1) * 128].T  # [128c, 32d]
                for r in range(2):
                    out[:, cc * 64 + r * 32: cc * 64 + (r + 1) * 32] = blk
            return _bf(out)

        wq = [rep2(w_qkv[32 * hh:32 * hh + 32]) for hh in (ha, hb)]
        wk = [rep2(w_qkv[128 + 32 * hh:128 + 32 * hh + 32]) for hh in (ha, hb)]
        wv = np.zeros((128, 194), np.float32)
        for cc in range(2):
            wv[:, cc * 97 + 0: cc * 97 + 32] = \
                w_qkv[256 + 32 * ha:256 + 32 * ha + 32, cc * 128:(cc + 1) * 128].T
            wv[:, cc * 97 + 64: cc * 97 + 96] = \
                w_qkv[256 + 32 * hb:256 + 32 * hb + 32, cc * 128:(cc + 1) * 128].T
        wo = np.concatenate(
            [w_out[:, 32 * ha:32 * ha + 32].T, w_out[:, 32 * hb:32 * hb + 32].T],
            axis=0)  # [64, 256]
        in_maps.append({
            "xh": xh, "wq0": wq[0], "wq1": wq[1], "wk0": wk[0], "wk1": wk[1],
            "wv": _bf(wv), "wo": _bf(wo),
        })
    return in_maps


def run_cores(in_maps, trace=False):
    nc = get_compiled()
    res = bass_utils.run_bass_kernel_spmd(
        nc, in_maps, core_ids=list(range(8)), trace=trace)
    return res


def assemble_output(results, b_out):
    b_out = np.asarray(b_out, np.float32)
    y = np.zeros((B, C, N), np.float32)
    for b in range(B):
        y[b, 0:128] = results[2 * b]["yh"].astype(np.float32)
        y[b, 128:256] = results[2 * b + 1]["yh"].astype(np.float32)
    y += b_out[None, :, None]
    return y.reshape(B, C, H, W)


def kernel(x, w_qkv, w_out, b_out):
    in_maps = prep_core_inputs(x, w_qkv, w_out)
    res = run_cores(in_maps)
    return assemble_output(res.results, b_out)



# revision 12
# speedup vs baseline: 3.2096x; 1.7909x over previous
"""ConvAttention TRN2 Bass kernel.

Sharding: 16 (batch, head) pairs over 8 cores -> each core handles one batch b
and a head-pair (heads 0,1 or 2,3). Each core computes a partial output
y_part = w_out[:, pair_slice] @ attn_out_pair  [256, 4096] bf16.

Host I/O over the axon tunnel dominates wall time (~40 MiB/s), so the wire
format is minimized: each core uploads only HALF of its batch's x (1 MiB bf16;
the pair AllGathers the full x on device), and downloads only its 128-channel
shard of the final y (1 MiB bf16) after an on-device ReduceScatter(add) over
the pair's partials. b_out is added on host.

Per-core pipeline (all SPMD-identical, different data):
  phase0: qkv projections (bf16 matmuls), q/k replicated x3 across PE row
          quadrants for tile_position packing; v transposed via PE into
          v_ext (ones column appended -> softmax denominator for free).
  phase1: per (head, i-chunk of 512): sim_T[j,i] = k^T q on PE (3-way row
          packing, K=32), exp on ScalarE (PSUM->SBUF bf16, SCALE folded),
          out_T[d,i] = v_ext^T p_T accumulated over j-tiles with 2-way
          column packing (even/odd j-tiles to col quadrants 0/64).
  phase2: per (head, i-half): PE-transpose out_ext (A+B accumulated in
          PSUM), reciprocal of denominator, per-partition broadcast mult,
          PE-transpose back -> out_norm [64, n] bf16.
  phase3: y = w_outT^T @ out_norm -> DRAM fp32.
"""

import numpy as np
import ml_dtypes

import concourse.bass as bass
import concourse.bacc as bacc
import concourse.mybir as mybir
import concourse.tile as tile
from concourse import bass_utils
from concourse.masks import make_identity

from concourse.dve_spec import (
    Spec, Src0, C0, C1, C2, One, sq,
    lower as _dve_lower, _has_src1,
)
import concourse.dve_ops as _dops
from concourse.dve_uop import DveOpSpec as _DveOpSpec

BF16 = mybir.dt.bfloat16
F32 = mybir.dt.float32
AF = mybir.ActivationFunctionType


def _exp8_ref(in0, in1, c0, c1, c2):
    x = np.asarray(in0, np.float32)
    t = (x * np.float32(c0)).astype(np.float32)
    y0 = ((np.float32(1.0) + t) + (t * t) * np.float32(c1)).astype(np.float32)
    y = (y0 * y0).astype(np.float32)
    y = (y * y).astype(np.float32)
    return (y * y).astype(np.float32)


def _register_exp8():
    # exp(s0*8*x) ~= ((1+t) + t^2*(1/2 + t/6))^8, t = s0*x.  8 ALU stages.
    name = "EXP8_ANT"
    for op in _dops.OPS:
        if op.name == name:
            return op
    t = Src0 * C0
    body = sq(sq(sq((One + t) + sq(t) * C1)))
    spec = Spec(body=body, reference=_exp8_ref)
    row = max(_dops._SUB_OPCODE_FOR_NAME.values()) + 1
    _dops._SUB_OPCODE_FOR_NAME[name] = row
    shas = {}
    for ver in ("v3", "v4"):
        try:
            uops = _dve_lower(spec, ver=ver)
            shas[ver] = _DveOpSpec(name=name, opcode=row, uops=uops,
                                   rd1_en=_has_src1(spec)).sha(ver)
        except Exception:
            pass
    op = _dops.DveOp(name, spec, subdim=False, uops_sha=shas)
    _dops.OPS.append(op)
    _dops.CUSTOM_DVE_SPECS[name] = spec
    return op


EXP8 = _register_exp8()
# softmax-exp groups routed to the Vector engine (rest go to ScalarE)
DVE_GROUPS = frozenset({1, 3, 5, 8, 10, 12, 14})

HEADS = 4
DIM_HEAD = 32
SCALE = DIM_HEAD ** (-0.5)
B, C, H, W = 4, 256, 64, 64
N = H * W            # 4096
NT = N // 128        # 32 j-tiles
IC = 512             # i-chunk
NIC = N // IC        # 8 i-chunks
NG = NT // 2  # 16 groups of 2 j-tiles (2-way PE row packing)


PAIRS = [[0, 1], [2, 3], [4, 5], [6, 7]]


def build_program(nc, tc, phases=3):
    """Emit the per-core program. DRAM tensor names are the in_map keys."""
    xh = nc.dram_tensor("xh", [256, N // 2], BF16, kind="ExternalInput").ap()
    wq0 = nc.dram_tensor("wq0", [128, 128], BF16, kind="ExternalInput").ap()
    wq1 = nc.dram_tensor("wq1", [128, 128], BF16, kind="ExternalInput").ap()
    wk0 = nc.dram_tensor("wk0", [128, 128], BF16, kind="ExternalInput").ap()
    wk1 = nc.dram_tensor("wk1", [128, 128], BF16, kind="ExternalInput").ap()
    wv = nc.dram_tensor("wv", [128, 194], BF16, kind="ExternalInput").ap()
    wo = nc.dram_tensor("wo", [64, 256], BF16, kind="ExternalInput").ap()
    yh = nc.dram_tensor("yh", [128, N], BF16, kind="ExternalOutput").ap()

    with (
        tc.tile_pool(name="singles", bufs=1) as singles,
        tc.tile_pool(name="ppool", bufs=16) as ppool,
        tc.tile_pool(name="opool", bufs=3) as opool,
        tc.tile_pool(name="mpool", bufs=2) as mpool,
        tc.tile_pool(name="ypool", bufs=2) as ypool,
        tc.tile_pool(name="dram", bufs=1, space="DRAM") as dram,
        tc.tile_pool(name="psum", bufs=2, space="PSUM") as psum,
    ):
        # AllGather x within batch pairs: each core uploads half of its
        # batch's x; xg = [half0 rows ; half1 rows], identical on the pair.
        xh_b = dram.tile([256, N // 2], BF16)
        xg = dram.tile([512, N // 2], BF16)
        nc.gpsimd.dma_start(out=xh_b[:], in_=xh)
        nc.gpsimd.collective_compute(
            "AllGather", mybir.AluOpType.bypass, replica_groups=PAIRS,
            ins=[xh_b[:].opt()], outs=[xg[:].opt()],
        )
        ypart = dram.tile([256, N], BF16)
        yred = dram.tile([128, N], BF16)
        ident_f = singles.tile([128, 128], F32)
        ident_b = singles.tile([128, 128], BF16)
        make_identity(nc, ident_f[:])
        make_identity(nc, ident_b[:])

        sb_wq = [singles.tile([128, 128], BF16, tag=f"wq{j}", name=f"sb_wq{j}") for j in range(2)]
        sb_wk = [singles.tile([128, 128], BF16, tag=f"wk{j}", name=f"sb_wk{j}") for j in range(2)]
        sb_wv = singles.tile([128, 194], BF16)
        sb_wo = singles.tile([64, 256], BF16)
        nc.sync.dma_start(out=sb_wq[0][:], in_=wq0)
        nc.sync.dma_start(out=sb_wq[1][:], in_=wq1)
        nc.sync.dma_start(out=sb_wk[0][:], in_=wk0)
        nc.sync.dma_start(out=sb_wk[1][:], in_=wk1)
        nc.sync.dma_start(out=sb_wv[:], in_=wv)
        nc.sync.dma_start(out=sb_wo[:], in_=wo)

        sb_x = [singles.tile([128, N], BF16, tag=f"x{cc}", name=f"sb_x{cc}") for cc in range(2)]
        for cc in range(2):
            for half in range(2):
                nc.sync.dma_start(
                    out=sb_x[cc][:, half * (N // 2):(half + 1) * (N // 2)],
                    in_=xg[half * 256 + cc * 128: half * 256 + (cc + 1) * 128, :],
                )

        # ---- phase 0: projections --------------------------------------
        q_rep = [singles.tile([64, N], BF16, tag=f"qr{j}", name=f"q_rep{j}") for j in range(2)]
        k_rep = [singles.tile([64, N], BF16, tag=f"kr{j}", name=f"k_rep{j}") for j in range(2)]
        v2 = singles.tile([97, N], BF16)

        NCH = [(i * 1024, 1024) for i in range(4)]
        projs = [
            (sb_wq[0], 64, q_rep[0][:]), (sb_wq[1], 64, q_rep[1][:]),
            (sb_wk[0], 64, k_rep[0][:]), (sb_wk[1], 64, k_rep[1][:]),
            (sb_wv, 97, v2[:]),
        ]
        for w_sb, m, dst in projs:
            for n0, nw in NCH:
                ps = psum.tile([128, 1024], F32, tag="sim")
                for s in range(nw // 512):
                    for cc in range(2):
                        nc.tensor.matmul(
                            ps[0:m, s * 512:(s + 1) * 512],
                            lhsT=w_sb[:, cc * m:(cc + 1) * m],
                            rhs=sb_x[cc][:, n0 + s * 512:n0 + (s + 1) * 512],
                            start=(cc == 0), stop=(cc == 1),
                        )
                nc.any.tensor_copy(dst[0:m, n0:n0 + nw], ps[0:m, 0:nw])
        # ones rows for the denominator column of v_ext
        nc.vector.memset(v2[32:33, :], 1.0)
        nc.vector.memset(v2[96:97, :], 1.0)

        # v_ext_all[:, jt*66 + 33h : +33] = [v_h^T | ones] for j-tile jt
        v_ext = singles.tile([128, NT * 98], BF16)
        for b8 in range(NT // 8):
            vt = psum.tile([128, 8 * 98], BF16, tag="sim")
            for s in range(8):
                jt = b8 * 8 + s
                nc.tensor.matmul(
                    vt[:, s * 98:s * 98 + 97],
                    lhsT=v2[0:97, jt * 128:(jt + 1) * 128],
                    rhs=ident_b[0:97, 0:97],
                    is_transpose=True,
                )
            nc.vector.tensor_copy(
                v_ext[:, b8 * 8 * 98:(b8 + 1) * 8 * 98]
                    .rearrange("p (s c) -> p s c", c=98)[:, :, 0:97],
                vt[:].rearrange("p (s c) -> p s c", c=98)[:, :, 0:97])

        # ---- phases 1-3 ------------------------------------------------
        on_sb = singles.tile([64, N], BF16)  # normalized attn out, both heads

        if phases == 0:
            dbg = singles.tile([128, N], BF16, name="dbg")
            nc.any.tensor_copy(dbg[0:64, 0:N], q_rep[0][0:64, :])
            nc.any.tensor_copy(dbg[64:128, 0:N // 2],
                               v_ext[0:64, 0:N // 2])
            nc.sync.dma_start(out=yh, in_=dbg[:])
            return

        for half in range(2):
            for h in range(2):
                oe = opool.tile([97, N // 2], F32, tag="oext")
                nc.vector.memset(oe[32:64, :], 0.0)
                for icl in range(NIC // 2):
                    ic0 = half * (N // 2) + icl * IC
                    # sim + exp for all 32 j-tiles at this i-chunk
                    p3s = []
                    for g in range(NG):
                        sp = psum.tile([128, 1024], F32, tag="sim")
                        for q in range(2):
                            jt = 2 * g + q
                            nc.tensor.matmul(
                                sp[:, q * 512:(q + 1) * 512],
                                lhsT=k_rep[h][32 * q:32 * q + 32,
                                              jt * 128:(jt + 1) * 128],
                                rhs=q_rep[h][32 * q:32 * q + 32, ic0:ic0 + IC],
                                start=True, stop=True,
                                tile_position=(32 * q, 0),
                            )
                        p3 = ppool.tile([128, 1024], BF16, tag="p3")
                        if g in DVE_GROUPS:
                            nc.vector._custom_dve(
                                EXP8, out=p3[:], in0=sp[:],
                                s0=SCALE / 8.0, s1=0.5, imm2=0.0)
                        else:
                            nc.scalar.activation(p3[:], sp[:], AF.Exp,
                                                 scale=SCALE)
                        p3s.append(p3)
                    # out matmul: accumulate over j-tiles; even j-tiles go to
                    # bank 0 rows 0-32, odd to bank 1 rows 64-96 (col packing)
                    op = psum.tile([97, 2 * IC], F32, tag="out", bufs=1)
                    for jt in range(NT):
                        g, q = jt // 2, jt % 2
                        r0 = 64 * q
                        nc.tensor.matmul(
                            op[r0:r0 + 33, q * IC:(q + 1) * IC],
                            lhsT=v_ext[:, jt * 98 + 64 * h:jt * 98 + 64 * h + 33],
                            rhs=p3s[g][:, q * 512:(q + 1) * 512],
                            start=(jt < 2), stop=(jt >= NT - 2),
                            tile_position=(0, r0),
                        )
                    icl0 = icl * IC
                    nc.vector.tensor_copy(oe[0:33, icl0:icl0 + IC],
                                          op[0:33, 0:IC])
                    nc.vector.tensor_copy(oe[64:97, icl0:icl0 + IC],
                                          op[64:97, IC:2 * IC])

                if phases == 1:
                    if half == 0 and h == 0:
                        dbg1 = singles.tile([97, N // 2], BF16, name="dbg1")
                        nc.vector.tensor_copy(dbg1[:], oe[:])
                        nc.sync.dma_start(out=yh[0:97, 0:N // 2], in_=dbg1[:])
                    continue

                # phase 2: transpose, normalize, transpose back
                outT = mpool.tile([128, 16 * 33], F32, tag="outT")
                for b4 in range(4):
                    tp = psum.tile([128, 4 * 98], F32, tag="small")
                    for s in range(4):
                        it = b4 * 4 + s
                        nc.tensor.matmul(
                            tp[:, s * 98:s * 98 + 97],
                            lhsT=oe[0:97, it * 128:(it + 1) * 128],
                            rhs=ident_f[0:97, 0:97],
                            is_transpose=True,
                        )
                    dst = outT[:, b4 * 132:(b4 + 1) * 132] \
                        .rearrange("p (s c) -> p s c", c=33)
                    tpv = tp[:].rearrange("p (s c) -> p s c", c=98)
                    nc.vector.tensor_copy(dst, tpv[:, :, 0:33])
                    nc.vector.tensor_add(dst, dst, tpv[:, :, 64:97])
                if phases in (2.05, 2.1):
                    continue
                outT_v = outT[:].rearrange("p (t c) -> p t c", c=33)
                recip = mpool.tile([128, 16], F32, tag="recip")
                nc.vector.reciprocal(recip[:], outT_v[:, :, 32:33])
                onT = mpool.tile([128, 512], BF16, tag="onT")
                for t in range(16):
                    nc.vector.tensor_scalar_mul(
                        onT[:, t * 32:(t + 1) * 32],
                        outT_v[:, t, 0:32],
                        recip[:, t:t + 1],
                    )
                if phases == 2.2:
                    continue
                for b4 in range(4):
                    tb = psum.tile([64, 512], BF16, tag="small")
                    for s in range(4):
                        it = b4 * 4 + s
                        nc.tensor.matmul(
                            tb[32 * h:32 * h + 32, s * 128:(s + 1) * 128],
                            lhsT=onT[:, it * 32:(it + 1) * 32],
                            rhs=ident_b[:, 0:128],
                            is_transpose=True,
                            tile_position=(0, 32 * h),
                        )
                    dst0 = half * (N // 2) + b4 * 512
                    nc.vector.tensor_copy(
                        on_sb[32 * h:32 * h + 32, dst0:dst0 + 512],
                        tb[32 * h:32 * h + 32, :],
                    )

            if phases < 3:
                continue
            # phase 3: output projection for this i-half
            for ot in range(2):
                ysb = ypool.tile([128, N // 2], BF16, tag="ysb")
                for icc in range(NIC // 2):
                    yp = psum.tile([128, IC], F32, tag="small")
                    s0 = half * (N // 2) + icc * IC
                    nc.tensor.matmul(
                        yp[:],
                        lhsT=sb_wo[:, ot * 128:(ot + 1) * 128],
                        rhs=on_sb[0:64, s0:s0 + IC],
                        start=True, stop=True,
                    )
                    nc.any.tensor_copy(ysb[:, icc * IC:(icc + 1) * IC], yp[:])
                nc.sync.dma_start(
                    out=ypart[ot * 128:(ot + 1) * 128,
                              half * (N // 2):(half + 1) * (N // 2)],
                    in_=ysb[:],
                )

        if phases >= 3:
            # sum the two per-pair partials on device; even core keeps
            # channels 0:128, odd core channels 128:256.
            nc.gpsimd.collective_compute(
                "ReduceScatter", mybir.AluOpType.add, replica_groups=PAIRS,
                ins=[ypart[:].opt()], outs=[yred[:].opt()],
            )
            nc.sync.dma_start(out=yh, in_=yred[:])


_CACHE = {}


def get_compiled(phases=3):
    key = ("nc", phases)
    if key not in _CACHE:
        nc = bacc.Bacc("TRN2", target_bir_lowering=False, debug=False,
                       num_devices=8)
        with tile.TileContext(nc) as tc:
            build_program(nc, tc, phases=phases)
        nc.compile()
        _CACHE[key] = nc
    return _CACHE[key]


def _bf(a):
    return np.ascontiguousarray(a.astype(ml_dtypes.bfloat16))


def prep_core_inputs(x, w_qkv, w_out):
    """Build the 8 per-core in_maps (host-side slicing/transposes)."""
    x = np.asarray(x, np.float32).reshape(B, C, N)
    w_qkv = np.asarray(w_qkv, np.float32)
    w_out = np.asarray(w_out, np.float32)
    in_maps = []
    for core in range(8):
        b, pair = core // 2, core % 2
        ha, hb = 2 * pair, 2 * pair + 1
        # this core uploads spatial half `pair` of batch b; the pair
        # AllGathers to reconstruct the full x on device.
        xh = _bf(x[b].reshape(2 * 128, N)[:, pair * (N // 2):(pair + 1) * (N // 2)])

        def rep2(rows):  # [32, 256] weight rows -> [128, 128] replicated x2
            out = np.zeros((128, 128), np.float32)
            for cc in range(2):
                blk = rows[:, cc * 128:(cc + 1) * 128].T  # [128c, 32d]
                for r in range(2):
                    out[:, cc * 64 + r * 32: cc * 64 + (r + 1) * 32] = blk
            return _bf(out)

        wq = [rep2(w_qkv[32 * hh:32 * hh + 32]) for hh in (ha, hb)]
        wk = [rep2(w_qkv[128 + 32 * hh:128 + 32 * hh + 32]) for hh in (ha, hb)]
        wv = np.zeros((128, 194), np.float32)
        for cc in range(2):
            wv[:, cc * 97 + 0: cc * 97 + 32] = \
                w_qkv[256 + 32 * ha:256 + 32 * ha + 32, cc * 128:(cc + 1) * 128].T
            wv[:, cc * 97 + 64: cc * 97 + 96] = \
                w_qkv[256 + 32 * hb:256 + 32 * hb + 32, cc * 128:(cc + 1) * 128].T
        wo = np.concatenate(
            [w_out[:, 32 * ha:32 * ha + 32].T, w_out[:, 32 * hb:32 * hb + 32].T],
            axis=0)  # [64, 256]
        in_maps.append({
            "xh": xh, "wq0": wq[0], "wq1": wq[1], "wk0": wk[0], "wk1": wk[1],
            "wv": _bf(wv), "wo": _bf(wo),
        })
    return in_maps


def _build_runner(nc, n_cores=8):
    """Cached equivalent of bass_utils.run_bass_kernel_spmd's execute path.

    run_bass_kernel_spmd -> run_bass_via_pjrt builds a fresh jax.jit closure
    on every call, so each invocation pays a full XLA retrace + compile
    round trip (~0.7s here) on top of the actual transfer + execute. This
    builds the identical shard_map program once and reuses it; every call
    still uploads inputs + donated zero outputs, executes the NEFF on all 8
    cores, and downloads the outputs.
    """
    import jax
    from concourse import bass2jax
    from jax.sharding import Mesh, PartitionSpec
    from jax.experimental.shard_map import shard_map

    bass2jax.install_neuronx_cc_hook()
    assert nc.dbg_addr is None and nc.partition_id_tensor is not None
    partition_name = nc.partition_id_tensor.name

    in_names, out_names, out_avals, zero_outs = [], [], [], []
    for alloc in nc.m.functions[0].allocations:
        if not isinstance(alloc, mybir.MemoryLocationSet):
            continue
        name = alloc.memorylocations[0].name
        if alloc.kind == "ExternalInput":
            if name != partition_name:
                in_names.append(name)
        elif alloc.kind == "ExternalOutput":
            shape = tuple(alloc.tensor_shape)
            dtype = mybir.dt.np(alloc.dtype)
            out_names.append(name)
            out_avals.append(jax.core.ShapedArray(shape, dtype))
            zero_outs.append(np.zeros((n_cores * shape[0], *shape[1:]), dtype))
    n_params = len(in_names)
    bind_in_names = tuple(in_names + out_names + [partition_name])
    donate = tuple(range(n_params, n_params + len(out_names)))

    def _body(*args):
        operands = list(args)
        operands.append(bass2jax.partition_id_tensor())
        outs = bass2jax._bass_exec_p.bind(
            *operands,
            out_avals=tuple(out_avals),
            in_names=bind_in_names,
            out_names=tuple(out_names),
            lowering_input_output_aliases=(),
            sim_require_finite=True,
            sim_require_nnan=True,
            nc=nc,
        )
        return tuple(outs)

    devices = jax.devices()[:n_cores]
    mesh = Mesh(np.asarray(devices), ("core",))
    in_specs = (PartitionSpec("core"),) * (n_params + len(out_names))
    out_specs = (PartitionSpec("core"),) * len(out_names)
    sharded = jax.jit(
        shard_map(_body, mesh=mesh, in_specs=in_specs, out_specs=out_specs,
                  check_rep=False),
        donate_argnums=donate, keep_unused=True,
    )

    def run(in_maps):
        concat_in = [
            np.concatenate([np.asarray(in_maps[c][nm]) for c in range(n_cores)],
                           axis=0)
            for nm in in_names
        ]
        out_arrs = sharded(*concat_in, *zero_outs)
        results = [
            {nm: np.asarray(out_arrs[i]).reshape(n_cores, *out_avals[i].shape)[c]
             for i, nm in enumerate(out_names)}
            for c in range(n_cores)
        ]
        return bass_utils.BassKernelResults(
            results=results, instructions_and_trace=None,
            profile_json=None, exec_time_ns=None)

    return run


def run_cores(in_maps, trace=False):
    nc = get_compiled()
    if trace:
        return bass_utils.run_bass_kernel_spmd(
            nc, in_maps, core_ids=list(range(8)), trace=True)
    key = "runner"
    if key not in _CACHE:
        _CACHE[key] = _build_runner(nc)
    return _CACHE[key](in_maps)


def assemble_output(results, b_out):
    b_out = np.asarray(b_out, np.float32)
    y = np.zeros((B, C, N), np.float32)
    for b in range(B):
        y[b, 0:128] = results[2 * b]["yh"].astype(np.float32)
        y[b, 128:256] = results[2 * b + 1]["yh"].astype(np.float32)
    y += b_out[None, :, None]
    return y.reshape(B, C, H, W)


def kernel(x, w_qkv, w_out, b_out):
    in_maps = prep_core_inputs(x, w_qkv, w_out)
    res = run_cores(in_maps)
    return assemble_output(res.results, b_out)



# revision 13
# speedup vs baseline: 3.7276x; 1.1614x over previous
"""ConvAttention TRN2 Bass kernel.

Sharding: 16 (batch, head) pairs over 8 cores -> each core handles one batch b
and a head-pair (heads 0,1 or 2,3). Each core computes a partial output
y_part = w_out[:, pair_slice] @ attn_out_pair  [256, 4096] bf16.

Host I/O over the axon tunnel dominates wall time (~40 MiB/s), so the wire
format is minimized: each core uploads only HALF of its batch's x (1 MiB bf16;
the pair AllGathers the full x on device), and downloads only its 128-channel
shard of the final y (1 MiB bf16) after an on-device ReduceScatter(add) over
the pair's partials. b_out is added on host.

Per-core pipeline (all SPMD-identical, different data):
  phase0: qkv projections (bf16 matmuls), q/k replicated x3 across PE row
          quadrants for tile_position packing; v transposed via PE into
          v_ext (ones column appended -> softmax denominator for free).
  phase1: per (head, i-chunk of 512): sim_T[j,i] = k^T q on PE (3-way row
          packing, K=32), exp on ScalarE (PSUM->SBUF bf16, SCALE folded),
          out_T[d,i] = v_ext^T p_T accumulated over j-tiles with 2-way
          column packing (even/odd j-tiles to col quadrants 0/64).
  phase2: per (head, i-half): PE-transpose out_ext (A+B accumulated in
          PSUM), reciprocal of denominator, per-partition broadcast mult,
          PE-transpose back -> out_norm [64, n] bf16.
  phase3: y = w_outT^T @ out_norm -> DRAM fp32.
"""

import numpy as np
import ml_dtypes

import concourse.bass as bass
import concourse.bacc as bacc
import concourse.mybir as mybir
import concourse.tile as tile
from concourse import bass_utils
from concourse.masks import make_identity

from concourse.dve_spec import (
    Spec, Src0, C0, C1, C2, One, sq,
    lower as _dve_lower, _has_src1,
)
import concourse.dve_ops as _dops
from concourse.dve_uop import DveOpSpec as _DveOpSpec

BF16 = mybir.dt.bfloat16
F32 = mybir.dt.float32
AF = mybir.ActivationFunctionType


def _exp8_ref(in0, in1, c0, c1, c2):
    x = np.asarray(in0, np.float32)
    t = (x * np.float32(c0)).astype(np.float32)
    y0 = ((np.float32(1.0) + t) + (t * t) * np.float32(c1)).astype(np.float32)
    y = (y0 * y0).astype(np.float32)
    y = (y * y).astype(np.float32)
    return (y * y).astype(np.float32)


def _register_exp8():
    # exp(s0*8*x) ~= ((1+t) + t^2*(1/2 + t/6))^8, t = s0*x.  8 ALU stages.
    name = "EXP8_ANT"
    for op in _dops.OPS:
        if op.name == name:
            return op
    t = Src0 * C0
    body = sq(sq(sq((One + t) + sq(t) * C1)))
    spec = Spec(body=body, reference=_exp8_ref)
    row = max(_dops._SUB_OPCODE_FOR_NAME.values()) + 1
    _dops._SUB_OPCODE_FOR_NAME[name] = row
    shas = {}
    for ver in ("v3", "v4"):
        try:
            uops = _dve_lower(spec, ver=ver)
            shas[ver] = _DveOpSpec(name=name, opcode=row, uops=uops,
                                   rd1_en=_has_src1(spec)).sha(ver)
        except Exception:
            pass
    op = _dops.DveOp(name, spec, subdim=False, uops_sha=shas)
    _dops.OPS.append(op)
    _dops.CUSTOM_DVE_SPECS[name] = spec
    return op


EXP8 = _register_exp8()
# softmax-exp groups routed to the Vector engine (rest go to ScalarE)
DVE_GROUPS = frozenset({1, 3, 5, 8, 10, 12, 14})

HEADS = 4
DIM_HEAD = 32
SCALE = DIM_HEAD ** (-0.5)
B, C, H, W = 4, 256, 64, 64
N = H * W            # 4096
NT = N // 128        # 32 j-tiles
IC = 512             # i-chunk
NIC = N // IC        # 8 i-chunks
NG = NT // 2  # 16 groups of 2 j-tiles (2-way PE row packing)


PAIRS = [[0, 1], [2, 3], [4, 5], [6, 7]]


def build_program(nc, tc, phases=3):
    """Emit the per-core program. DRAM tensor names are the in_map keys."""
    xh = nc.dram_tensor("xh", [256, N // 2], BF16, kind="ExternalInput").ap()
    wq0 = nc.dram_tensor("wq0", [128, 128], BF16, kind="ExternalInput").ap()
    wq1 = nc.dram_tensor("wq1", [128, 128], BF16, kind="ExternalInput").ap()
    wk0 = nc.dram_tensor("wk0", [128, 128], BF16, kind="ExternalInput").ap()
    wk1 = nc.dram_tensor("wk1", [128, 128], BF16, kind="ExternalInput").ap()
    wv = nc.dram_tensor("wv", [128, 194], BF16, kind="ExternalInput").ap()
    wo = nc.dram_tensor("wo", [64, 256], BF16, kind="ExternalInput").ap()
    yh = nc.dram_tensor("yh", [128, N], BF16, kind="ExternalOutput").ap()

    with (
        tc.tile_pool(name="singles", bufs=1) as singles,
        tc.tile_pool(name="ppool", bufs=16) as ppool,
        tc.tile_pool(name="opool", bufs=3) as opool,
        tc.tile_pool(name="mpool", bufs=2) as mpool,
        tc.tile_pool(name="ypool", bufs=2) as ypool,
        tc.tile_pool(name="dram", bufs=1, space="DRAM") as dram,
        tc.tile_pool(name="psum", bufs=2, space="PSUM") as psum,
    ):
        # AllGather x within batch pairs: each core uploads half of its
        # batch's x; xg = [half0 rows ; half1 rows], identical on the pair.
        xh_b = dram.tile([256, N // 2], BF16)
        xg = dram.tile([512, N // 2], BF16)
        nc.gpsimd.dma_start(out=xh_b[:], in_=xh)
        nc.gpsimd.collective_compute(
            "AllGather", mybir.AluOpType.bypass, replica_groups=PAIRS,
            ins=[xh_b[:].opt()], outs=[xg[:].opt()],
        )
        ypart = dram.tile([256, N], BF16)
        yred = dram.tile([128, N], BF16)
        ident_f = singles.tile([128, 128], F32)
        ident_b = singles.tile([128, 128], BF16)
        make_identity(nc, ident_f[:])
        make_identity(nc, ident_b[:])

        sb_wq = [singles.tile([128, 128], BF16, tag=f"wq{j}", name=f"sb_wq{j}") for j in range(2)]
        sb_wk = [singles.tile([128, 128], BF16, tag=f"wk{j}", name=f"sb_wk{j}") for j in range(2)]
        sb_wv = singles.tile([128, 194], BF16)
        sb_wo = singles.tile([64, 256], BF16)
        nc.sync.dma_start(out=sb_wq[0][:], in_=wq0)
        nc.sync.dma_start(out=sb_wq[1][:], in_=wq1)
        nc.sync.dma_start(out=sb_wk[0][:], in_=wk0)
        nc.sync.dma_start(out=sb_wk[1][:], in_=wk1)
        nc.sync.dma_start(out=sb_wv[:], in_=wv)
        nc.sync.dma_start(out=sb_wo[:], in_=wo)

        sb_x = [singles.tile([128, N], BF16, tag=f"x{cc}", name=f"sb_x{cc}") for cc in range(2)]
        for cc in range(2):
            for half in range(2):
                nc.sync.dma_start(
                    out=sb_x[cc][:, half * (N // 2):(half + 1) * (N // 2)],
                    in_=xg[half * 256 + cc * 128: half * 256 + (cc + 1) * 128, :],
                )

        # ---- phase 0: projections --------------------------------------
        q_rep = [singles.tile([64, N], BF16, tag=f"qr{j}", name=f"q_rep{j}") for j in range(2)]
        k_rep = [singles.tile([64, N], BF16, tag=f"kr{j}", name=f"k_rep{j}") for j in range(2)]
        v2 = singles.tile([97, N], BF16)

        NCH = [(i * 1024, 1024) for i in range(4)]
        projs = [
            (sb_wq[0], 64, q_rep[0][:]), (sb_wq[1], 64, q_rep[1][:]),
            (sb_wk[0], 64, k_rep[0][:]), (sb_wk[1], 64, k_rep[1][:]),
            (sb_wv, 97, v2[:]),
        ]
        for w_sb, m, dst in projs:
            for n0, nw in NCH:
                ps = psum.tile([128, 1024], F32, tag="sim")
                for s in range(nw // 512):
                    for cc in range(2):
                        nc.tensor.matmul(
                            ps[0:m, s * 512:(s + 1) * 512],
                            lhsT=w_sb[:, cc * m:(cc + 1) * m],
                            rhs=sb_x[cc][:, n0 + s * 512:n0 + (s + 1) * 512],
                            start=(cc == 0), stop=(cc == 1),
                        )
                nc.any.tensor_copy(dst[0:m, n0:n0 + nw], ps[0:m, 0:nw])
        # ones rows for the denominator column of v_ext
        nc.vector.memset(v2[32:33, :], 1.0)
        nc.vector.memset(v2[96:97, :], 1.0)

        # v_ext_all[:, jt*66 + 33h : +33] = [v_h^T | ones] for j-tile jt
        v_ext = singles.tile([128, NT * 98], BF16)
        for b8 in range(NT // 8):
            vt = psum.tile([128, 8 * 98], BF16, tag="sim")
            for s in range(8):
                jt = b8 * 8 + s
                nc.tensor.matmul(
                    vt[:, s * 98:s * 98 + 97],
                    lhsT=v2[0:97, jt * 128:(jt + 1) * 128],
                    rhs=ident_b[0:97, 0:97],
                    is_transpose=True,
                )
            nc.vector.tensor_copy(
                v_ext[:, b8 * 8 * 98:(b8 + 1) * 8 * 98]
                    .rearrange("p (s c) -> p s c", c=98)[:, :, 0:97],
                vt[:].rearrange("p (s c) -> p s c", c=98)[:, :, 0:97])

        # ---- phases 1-3 ------------------------------------------------
        on_sb = singles.tile([64, N], BF16)  # normalized attn out, both heads

        if phases == 0:
            dbg = singles.tile([128, N], BF16, name="dbg")
            nc.any.tensor_copy(dbg[0:64, 0:N], q_rep[0][0:64, :])
            nc.any.tensor_copy(dbg[64:128, 0:N // 2],
                               v_ext[0:64, 0:N // 2])
            nc.sync.dma_start(out=yh, in_=dbg[:])
            return

        for half in range(2):
            for h in range(2):
                oe = opool.tile([97, N // 2], F32, tag="oext")
                nc.vector.memset(oe[32:64, :], 0.0)
                for icl in range(NIC // 2):
                    ic0 = half * (N // 2) + icl * IC
                    # sim + exp for all 32 j-tiles at this i-chunk
                    p3s = []
                    for g in range(NG):
                        sp = psum.tile([128, 1024], F32, tag="sim")
                        for q in range(2):
                            jt = 2 * g + q
                            nc.tensor.matmul(
                                sp[:, q * 512:(q + 1) * 512],
                                lhsT=k_rep[h][32 * q:32 * q + 32,
                                              jt * 128:(jt + 1) * 128],
                                rhs=q_rep[h][32 * q:32 * q + 32, ic0:ic0 + IC],
                                start=True, stop=True,
                                tile_position=(32 * q, 0),
                            )
                        p3 = ppool.tile([128, 1024], BF16, tag="p3")
                        if g in DVE_GROUPS:
                            nc.vector._custom_dve(
                                EXP8, out=p3[:], in0=sp[:],
                                s0=SCALE / 8.0, s1=0.5, imm2=0.0)
                        else:
                            nc.scalar.activation(p3[:], sp[:], AF.Exp,
                                                 scale=SCALE)
                        p3s.append(p3)
                    # out matmul: accumulate over j-tiles; even j-tiles go to
                    # bank 0 rows 0-32, odd to bank 1 rows 64-96 (col packing)
                    op = psum.tile([97, 2 * IC], F32, tag="out", bufs=1)
                    for jt in range(NT):
                        g, q = jt // 2, jt % 2
                        r0 = 64 * q
                        nc.tensor.matmul(
                            op[r0:r0 + 33, q * IC:(q + 1) * IC],
                            lhsT=v_ext[:, jt * 98 + 64 * h:jt * 98 + 64 * h + 33],
                            rhs=p3s[g][:, q * 512:(q + 1) * 512],
                            start=(jt < 2), stop=(jt >= NT - 2),
                            tile_position=(0, r0),
                        )
                    icl0 = icl * IC
                    nc.vector.tensor_copy(oe[0:33, icl0:icl0 + IC],
                                          op[0:33, 0:IC])
                    nc.vector.tensor_copy(oe[64:97, icl0:icl0 + IC],
                                          op[64:97, IC:2 * IC])

                if phases == 1:
                    if half == 0 and h == 0:
                        dbg1 = singles.tile([97, N // 2], BF16, name="dbg1")
                        nc.vector.tensor_copy(dbg1[:], oe[:])
                        nc.sync.dma_start(out=yh[0:97, 0:N // 2], in_=dbg1[:])
                    continue

                # phase 2: transpose, normalize, transpose back
                outT = mpool.tile([128, 16 * 33], F32, tag="outT")
                for b4 in range(4):
                    tp = psum.tile([128, 4 * 98], F32, tag="small")
                    for s in range(4):
                        it = b4 * 4 + s
                        nc.tensor.matmul(
                            tp[:, s * 98:s * 98 + 97],
                            lhsT=oe[0:97, it * 128:(it + 1) * 128],
                            rhs=ident_f[0:97, 0:97],
                            is_transpose=True,
                        )
                    dst = outT[:, b4 * 132:(b4 + 1) * 132] \
                        .rearrange("p (s c) -> p s c", c=33)
                    tpv = tp[:].rearrange("p (s c) -> p s c", c=98)
                    nc.vector.tensor_copy(dst, tpv[:, :, 0:33])
                    nc.vector.tensor_add(dst, dst, tpv[:, :, 64:97])
                if phases in (2.05, 2.1):
                    continue
                outT_v = outT[:].rearrange("p (t c) -> p t c", c=33)
                recip = mpool.tile([128, 16], F32, tag="recip")
                nc.vector.reciprocal(recip[:], outT_v[:, :, 32:33])
                onT = mpool.tile([128, 512], BF16, tag="onT")
                for t in range(16):
                    nc.vector.tensor_scalar_mul(
                        onT[:, t * 32:(t + 1) * 32],
                        outT_v[:, t, 0:32],
                        recip[:, t:t + 1],
                    )
                if phases == 2.2:
                    continue
                for b4 in range(4):
                    tb = psum.tile([64, 512], BF16, tag="small")
                    for s in range(4):
                        it = b4 * 4 + s
                        nc.tensor.matmul(
                            tb[32 * h:32 * h + 32, s * 128:(s + 1) * 128],
                            lhsT=onT[:, it * 32:(it + 1) * 32],
                            rhs=ident_b[:, 0:128],
                            is_transpose=True,
                            tile_position=(0, 32 * h),
                        )
                    dst0 = half * (N // 2) + b4 * 512
                    nc.vector.tensor_copy(
                        on_sb[32 * h:32 * h + 32, dst0:dst0 + 512],
                        tb[32 * h:32 * h + 32, :],
                    )

            if phases < 3:
                continue
            # phase 3: output projection for this i-half
            for ot in range(2):
                ysb = ypool.tile([128, N // 2], BF16, tag="ysb")
                for icc in range(NIC // 2):
                    yp = psum.tile([128, IC], F32, tag="small")
                    s0 = half * (N // 2) + icc * IC
                    nc.tensor.matmul(
                        yp[:],
                        lhsT=sb_wo[:, ot * 128:(ot + 1) * 128],
                        rhs=on_sb[0:64, s0:s0 + IC],
                        start=True, stop=True,
                    )
                    nc.any.tensor_copy(ysb[:, icc * IC:(icc + 1) * IC], yp[:])
                nc.sync.dma_start(
                    out=ypart[ot * 128:(ot + 1) * 128,
                              half * (N // 2):(half + 1) * (N // 2)],
                    in_=ysb[:],
                )

        if phases >= 3:
            # sum the two per-pair partials on device; even core keeps
            # channels 0:128, odd core channels 128:256.
            nc.gpsimd.collective_compute(
                "ReduceScatter", mybir.AluOpType.add, replica_groups=PAIRS,
                ins=[ypart[:].opt()], outs=[yred[:].opt()],
            )
            nc.sync.dma_start(out=yh, in_=yred[:])


_CACHE = {}


def get_compiled(phases=3):
    key = ("nc", phases)
    if key not in _CACHE:
        nc = bacc.Bacc("TRN2", target_bir_lowering=False, debug=False,
                       num_devices=8)
        with tile.TileContext(nc) as tc:
            build_program(nc, tc, phases=phases)
        nc.compile()
        _CACHE[key] = nc
    return _CACHE[key]


def _bf(a):
    return np.ascontiguousarray(a.astype(ml_dtypes.bfloat16))


def prep_core_inputs(x, w_qkv, w_out):
    """Build the 8 per-core in_maps (host-side slicing/transposes)."""
    x = np.asarray(x, np.float32).reshape(B, C, N)
    w_qkv = np.asarray(w_qkv, np.float32)
    w_out = np.asarray(w_out, np.float32)
    in_maps = []
    for core in range(8):
        b, pair = core // 2, core % 2
        ha, hb = 2 * pair, 2 * pair + 1
        # this core uploads spatial half `pair` of batch b; the pair
        # AllGathers to reconstruct the full x on device.
        xh = _bf(x[b].reshape(2 * 128, N)[:, pair * (N // 2):(pair + 1) * (N // 2)])

        def rep2(rows):  # [32, 256] weight rows -> [128, 128] replicated x2
            out = np.zeros((128, 128), np.float32)
            for cc in range(2):
                blk = rows[:, cc * 128:(cc + 1) * 128].T  # [128c, 32d]
                for r in range(2):
                    out[:, cc * 64 + r * 32: cc * 64 + (r + 1) * 32] = blk
            return _bf(out)

        wq = [rep2(w_qkv[32 * hh:32 * hh + 32]) for hh in (ha, hb)]
        wk = [rep2(w_qkv[128 + 32 * hh:128 + 32 * hh + 32]) for hh in (ha, hb)]
        wv = np.zeros((128, 194), np.float32)
        for cc in range(2):
            wv[:, cc * 97 + 0: cc * 97 + 32] = \
                w_qkv[256 + 32 * ha:256 + 32 * ha + 32, cc * 128:(cc + 1) * 128].T
            wv[:, cc * 97 + 64: cc * 97 + 96] = \
                w_qkv[256 + 32 * hb:256 + 32 * hb + 32, cc * 128:(cc + 1) * 128].T
        wo = np.concatenate(
            [w_out[:, 32 * ha:32 * ha + 32].T, w_out[:, 32 * hb:32 * hb + 32].T],
            axis=0)  # [64, 256]
        in_maps.append({
            "xh": xh, "wq0": wq[0], "wq1": wq[1], "wk0": wk[0], "wk1": wk[1],
            "wv": _bf(wv), "wo": _bf(wo),
        })
    return in_maps


def _build_runner(nc, n_cores=8):
    """Cached equivalent of bass_utils.run_bass_kernel_spmd's execute path.

    run_bass_kernel_spmd -> run_bass_via_pjrt builds a fresh jax.jit closure
    on every call, so each invocation pays a full XLA retrace + compile
    round trip (~0.7s here) on top of the actual transfer + execute. This
    builds the identical shard_map program once and reuses it; every call
    still uploads inputs + donated zero outputs, executes the NEFF on all 8
    cores, and downloads the outputs.
    """
    import jax
    from concourse import bass2jax
    from jax.sharding import Mesh, PartitionSpec
    from jax.experimental.shard_map import shard_map

    bass2jax.install_neuronx_cc_hook()
    assert nc.dbg_addr is None and nc.partition_id_tensor is not None
    partition_name = nc.partition_id_tensor.name

    in_names, out_names, out_avals, zero_outs = [], [], [], []
    for alloc in nc.m.functions[0].allocations:
        if not isinstance(alloc, mybir.MemoryLocationSet):
            continue
        name = alloc.memorylocations[0].name
        if alloc.kind == "ExternalInput":
            if name != partition_name:
                in_names.append(name)
        elif alloc.kind == "ExternalOutput":
            shape = tuple(alloc.tensor_shape)
            dtype = mybir.dt.np(alloc.dtype)
            out_names.append(name)
            out_avals.append(jax.core.ShapedArray(shape, dtype))
            zero_outs.append(np.zeros((n_cores * shape[0], *shape[1:]), dtype))
    n_params = len(in_names)
    bind_in_names = tuple(in_names + out_names + [partition_name])
    donate = tuple(range(n_params, n_params + len(out_names)))

    def _body(*args):
        operands = list(args)
        operands.append(bass2jax.partition_id_tensor())
        outs = bass2jax._bass_exec_p.bind(
            *operands,
            out_avals=tuple(out_avals),
            in_names=bind_in_names,
            out_names=tuple(out_names),
            lowering_input_output_aliases=(),
            sim_require_finite=True,
            sim_require_nnan=True,
            nc=nc,
        )
        return tuple(outs)

    devices = jax.devices()[:n_cores]
    mesh = Mesh(np.asarray(devices), ("core",))
    in_specs = (PartitionSpec("core"),) * (n_params + len(out_names))
    out_specs = (PartitionSpec("core"),) * len(out_names)
    sharded = jax.jit(
        shard_map(_body, mesh=mesh, in_specs=in_specs, out_specs=out_specs,
                  check_rep=False),
        donate_argnums=donate, keep_unused=True,
    )

    # The kernel fully overwrites its ExternalOutput (one whole-tensor DMA),
    # so the donated output-scratch operand's *content* is irrelevant; only
    # its shape/sharding matter. First call donates host zeros; later calls
    # donate the previous call's device-resident output, avoiding an 8 MiB
    # host->device upload per invocation.
    scratch = list(zero_outs)

    def run(in_maps):
        concat_in = [
            np.concatenate([np.asarray(in_maps[c][nm]) for c in range(n_cores)],
                           axis=0)
            for nm in in_names
        ]
        out_arrs = sharded(*concat_in, *scratch)
        results = [
            {nm: np.asarray(out_arrs[i]).reshape(n_cores, *out_avals[i].shape)[c]
             for i, nm in enumerate(out_names)}
            for c in range(n_cores)
        ]
        scratch[:] = list(out_arrs)
        return bass_utils.BassKernelResults(
            results=results, instructions_and_trace=None,
            profile_json=None, exec_time_ns=None)

    return run


def run_cores(in_maps, trace=False):
    nc = get_compiled()
    if trace:
        return bass_utils.run_bass_kernel_spmd(
            nc, in_maps, core_ids=list(range(8)), trace=True)
    key = "runner"
    if key not in _CACHE:
        _CACHE[key] = _build_runner(nc)
    return _CACHE[key](in_maps)


def assemble_output(results, b_out):
    b_out = np.asarray(b_out, np.float32)
    y = np.zeros((B, C, N), np.float32)
    for b in range(B):
        y[b, 0:128] = results[2 * b]["yh"].astype(np.float32)
        y[b, 128:256] = results[2 * b + 1]["yh"].astype(np.float32)
    y += b_out[None, :, None]
    return y.reshape(B, C, H, W)


def kernel(x, w_qkv, w_out, b_out):
    in_maps = prep_core_inputs(x, w_qkv, w_out)
    res = run_cores(in_maps)
    return assemble_output(res.results, b_out)



# revision 23
# speedup vs baseline: 6.0660x; 1.6273x over previous
"""ConvAttention TRN2 Bass kernel.

Sharding: 16 (batch, head) pairs over 8 cores -> each core handles one batch b
and a head-pair (heads 0,1 or 2,3). Each core computes a partial output
y_part = w_out[:, pair_slice] @ attn_out_pair  [256, 4096] bf16.

Host I/O over the axon tunnel dominates wall time (~40 MiB/s), so the wire
format is minimized: each core uploads only HALF of its batch's x (1 MiB bf16;
the pair AllGathers the full x on device), and downloads only its 128-channel
shard of the final y (1 MiB bf16) after an on-device ReduceScatter(add) over
the pair's partials. b_out is added on host.

Per-core pipeline (all SPMD-identical, different data):
  phase0: qkv projections (bf16 matmuls), q/k replicated x3 across PE row
          quadrants for tile_position packing; v transposed via PE into
          v_ext (ones column appended -> softmax denominator for free).
  phase1: per (head, i-chunk of 512): sim_T[j,i] = k^T q on PE (3-way row
          packing, K=32), exp on ScalarE (PSUM->SBUF bf16, SCALE folded),
          out_T[d,i] = v_ext^T p_T accumulated over j-tiles with 2-way
          column packing (even/odd j-tiles to col quadrants 0/64).
  phase2: per (head, i-half): PE-transpose out_ext (A+B accumulated in
          PSUM), reciprocal of denominator, per-partition broadcast mult,
          PE-transpose back -> out_norm [64, n] bf16.
  phase3: y = w_outT^T @ out_norm -> DRAM fp32.
"""

import numpy as np
import ml_dtypes

import concourse.bass as bass
import concourse.bacc as bacc
import concourse.mybir as mybir
import concourse.tile as tile
from concourse import bass_utils
from concourse.masks import make_identity

from concourse.dve_spec import (
    Spec, Src0, C0, C1, C2, One, sq,
    lower as _dve_lower, _has_src1,
)
import concourse.dve_ops as _dops
from concourse.dve_uop import DveOpSpec as _DveOpSpec

BF16 = mybir.dt.bfloat16
F32 = mybir.dt.float32
I8 = mybir.dt.int8
AF = mybir.ActivationFunctionType


def _exp8_ref(in0, in1, c0, c1, c2):
    x = np.asarray(in0, np.float32)
    t = (x * np.float32(c0)).astype(np.float32)
    y0 = ((np.float32(1.0) + t) + (t * t) * np.float32(c1)).astype(np.float32)
    y = (y0 * y0).astype(np.float32)
    y = (y * y).astype(np.float32)
    return (y * y).astype(np.float32)


def _register_exp8():
    # exp(s0*8*x) ~= ((1+t) + t^2*(1/2 + t/6))^8, t = s0*x.  8 ALU stages.
    name = "EXP8_ANT"
    for op in _dops.OPS:
        if op.name == name:
            return op
    t = Src0 * C0
    body = sq(sq(sq((One + t) + sq(t) * C1)))
    spec = Spec(body=body, reference=_exp8_ref)
    row = max(_dops._SUB_OPCODE_FOR_NAME.values()) + 1
    _dops._SUB_OPCODE_FOR_NAME[name] = row
    shas = {}
    for ver in ("v3", "v4"):
        try:
            uops = _dve_lower(spec, ver=ver)
            shas[ver] = _DveOpSpec(name=name, opcode=row, uops=uops,
                                   rd1_en=_has_src1(spec)).sha(ver)
        except Exception:
            pass
    op = _dops.DveOp(name, spec, subdim=False, uops_sha=shas)
    _dops.OPS.append(op)
    _dops.CUSTOM_DVE_SPECS[name] = spec
    return op


EXP8 = _register_exp8()
# softmax-exp groups routed to the Vector engine (rest go to ScalarE)
DVE_GROUPS = frozenset({1, 3, 5, 8, 10, 12, 14})

HEADS = 4
DIM_HEAD = 32
SCALE = DIM_HEAD ** (-0.5)
B, C, H, W = 4, 256, 64, 64
N = H * W            # 4096
NT = N // 128        # 32 j-tiles
IC = 512             # i-chunk
NIC = N // IC        # 8 i-chunks
NG = NT // 2  # 16 groups of 2 j-tiles (2-way PE row packing)


PAIRS = [[0, 1], [2, 3], [4, 5], [6, 7]]


def build_program(nc, tc, phases=3):
    """Emit the per-core program. DRAM tensor names are the in_map keys."""
    xh = nc.dram_tensor("xh", [256, N // 2], I8, kind="ExternalInput").ap()
    wq0 = nc.dram_tensor("wq0", [128, 128], BF16, kind="ExternalInput").ap()
    wq1 = nc.dram_tensor("wq1", [128, 128], BF16, kind="ExternalInput").ap()
    wk0 = nc.dram_tensor("wk0", [128, 128], BF16, kind="ExternalInput").ap()
    wk1 = nc.dram_tensor("wk1", [128, 128], BF16, kind="ExternalInput").ap()
    wv = nc.dram_tensor("wv", [128, 194], BF16, kind="ExternalInput").ap()
    on_out = nc.dram_tensor("on", [64, N], BF16, kind="ExternalOutput").ap()

    with (
        tc.tile_pool(name="singles", bufs=1) as singles,
        tc.tile_pool(name="ppool", bufs=16) as ppool,
        tc.tile_pool(name="opool", bufs=3) as opool,
        tc.tile_pool(name="mpool", bufs=2) as mpool,
        tc.tile_pool(name="ypool", bufs=2) as ypool,
        tc.tile_pool(name="dram", bufs=1, space="DRAM") as dram,
        tc.tile_pool(name="psum", bufs=2, space="PSUM") as psum,
    ):
        # AllGather x within batch pairs: each core uploads half of its
        # batch's x; xg = [half0 rows ; half1 rows], identical on the pair.
        xh_b = dram.tile([256, N // 2], I8)
        xg = dram.tile([512, N // 2], I8)
        nc.gpsimd.dma_start(out=xh_b[:], in_=xh)
        nc.gpsimd.collective_compute(
            "AllGather", mybir.AluOpType.bypass, replica_groups=PAIRS,
            ins=[xh_b[:].opt()], outs=[xg[:].opt()],
        )
        ident_f = singles.tile([128, 128], F32)
        ident_b = singles.tile([128, 128], BF16)
        make_identity(nc, ident_f[:])
        make_identity(nc, ident_b[:])

        sb_wq = [singles.tile([128, 128], BF16, tag=f"wq{j}", name=f"sb_wq{j}") for j in range(2)]
        sb_wk = [singles.tile([128, 128], BF16, tag=f"wk{j}", name=f"sb_wk{j}") for j in range(2)]
        sb_wv = singles.tile([128, 194], BF16)
        nc.sync.dma_start(out=sb_wq[0][:], in_=wq0)
        nc.sync.dma_start(out=sb_wq[1][:], in_=wq1)
        nc.sync.dma_start(out=sb_wk[0][:], in_=wk0)
        nc.sync.dma_start(out=sb_wk[1][:], in_=wk1)
        nc.sync.dma_start(out=sb_wv[:], in_=wv)

        # x arrives int8 (per-channel scales folded into the qkv weights on
        # host); DMA the gathered int8, convert to bf16 for the PE matmuls.
        sb_x = [singles.tile([128, N], BF16, tag=f"x{cc}", name=f"sb_x{cc}") for cc in range(2)]
        sxq = [singles.tile([128, N], I8, tag=f"xq{cc}", name=f"sxq{cc}") for cc in range(2)]
        for cc in range(2):
            for half in range(2):
                nc.sync.dma_start(
                    out=sxq[cc][:, half * (N // 2):(half + 1) * (N // 2)],
                    in_=xg[half * 256 + cc * 128: half * 256 + (cc + 1) * 128, :],
                )
                nc.any.tensor_copy(
                    sb_x[cc][:, half * (N // 2):(half + 1) * (N // 2)],
                    sxq[cc][:, half * (N // 2):(half + 1) * (N // 2)])

        # ---- phase 0: projections --------------------------------------
        q_rep = [singles.tile([64, N], BF16, tag=f"qr{j}", name=f"q_rep{j}") for j in range(2)]
        k_rep = [singles.tile([64, N], BF16, tag=f"kr{j}", name=f"k_rep{j}") for j in range(2)]
        v2 = singles.tile([97, N], BF16)

        NCH = [(i * 1024, 1024) for i in range(4)]
        projs = [
            (sb_wq[0], 64, q_rep[0][:]), (sb_wq[1], 64, q_rep[1][:]),
            (sb_wk[0], 64, k_rep[0][:]), (sb_wk[1], 64, k_rep[1][:]),
            (sb_wv, 97, v2[:]),
        ]
        for w_sb, m, dst in projs:
            for n0, nw in NCH:
                ps = psum.tile([128, 1024], F32, tag="sim")
                for s in range(nw // 512):
                    for cc in range(2):
                        nc.tensor.matmul(
                            ps[0:m, s * 512:(s + 1) * 512],
                            lhsT=w_sb[:, cc * m:(cc + 1) * m],
                            rhs=sb_x[cc][:, n0 + s * 512:n0 + (s + 1) * 512],
                            start=(cc == 0), stop=(cc == 1),
                        )
                nc.any.tensor_copy(dst[0:m, n0:n0 + nw], ps[0:m, 0:nw])
        # ones rows for the denominator column of v_ext
        nc.vector.memset(v2[32:33, :], 1.0)
        nc.vector.memset(v2[96:97, :], 1.0)

        # v_ext_all[:, jt*66 + 33h : +33] = [v_h^T | ones] for j-tile jt
        v_ext = singles.tile([128, NT * 98], BF16)
        for b8 in range(NT // 8):
            vt = psum.tile([128, 8 * 98], BF16, tag="sim")
            for s in range(8):
                jt = b8 * 8 + s
                nc.tensor.matmul(
                    vt[:, s * 98:s * 98 + 97],
                    lhsT=v2[0:97, jt * 128:(jt + 1) * 128],
                    rhs=ident_b[0:97, 0:97],
                    is_transpose=True,
                )
            nc.vector.tensor_copy(
                v_ext[:, b8 * 8 * 98:(b8 + 1) * 8 * 98]
                    .rearrange("p (s c) -> p s c", c=98)[:, :, 0:97],
                vt[:].rearrange("p (s c) -> p s c", c=98)[:, :, 0:97])

        # ---- phases 1-3 ------------------------------------------------
        on_sb = singles.tile([64, N], BF16)  # normalized attn out, both heads

        if phases == 0:
            dbg = singles.tile([128, N], BF16, name="dbg")
            nc.any.tensor_copy(dbg[0:64, 0:N], q_rep[0][0:64, :])
            nc.any.tensor_copy(dbg[64:128, 0:N // 2],
                               v_ext[0:64, 0:N // 2])
            nc.sync.dma_start(out=on_out, in_=dbg[0:64, :])
            return

        for half in range(2):
            for h in range(2):
                oe = opool.tile([97, N // 2], F32, tag="oext")
                nc.vector.memset(oe[32:64, :], 0.0)
                for icl in range(NIC // 2):
                    ic0 = half * (N // 2) + icl * IC
                    # sim + exp for all 32 j-tiles at this i-chunk
                    p3s = []
                    for g in range(NG):
                        sp = psum.tile([128, 1024], F32, tag="sim")
                        for q in range(2):
                            jt = 2 * g + q
                            nc.tensor.matmul(
                                sp[:, q * 512:(q + 1) * 512],
                                lhsT=k_rep[h][32 * q:32 * q + 32,
                                              jt * 128:(jt + 1) * 128],
                                rhs=q_rep[h][32 * q:32 * q + 32, ic0:ic0 + IC],
                                start=True, stop=True,
                                tile_position=(32 * q, 0),
                            )
                        p3 = ppool.tile([128, 1024], BF16, tag="p3")
                        if g in DVE_GROUPS:
                            nc.vector._custom_dve(
                                EXP8, out=p3[:], in0=sp[:],
                                s0=SCALE / 8.0, s1=0.5, imm2=0.0)
                        else:
                            nc.scalar.activation(p3[:], sp[:], AF.Exp,
                                                 scale=SCALE)
                        p3s.append(p3)
                    # out matmul: accumulate over j-tiles; even j-tiles go to
                    # bank 0 rows 0-32, odd to bank 1 rows 64-96 (col packing)
                    op = psum.tile([97, 2 * IC], F32, tag="out", bufs=1)
                    for jt in range(NT):
                        g, q = jt // 2, jt % 2
                        r0 = 64 * q
                        nc.tensor.matmul(
                            op[r0:r0 + 33, q * IC:(q + 1) * IC],
                            lhsT=v_ext[:, jt * 98 + 64 * h:jt * 98 + 64 * h + 33],
                            rhs=p3s[g][:, q * 512:(q + 1) * 512],
                            start=(jt < 2), stop=(jt >= NT - 2),
                            tile_position=(0, r0),
                        )
                    icl0 = icl * IC
                    nc.vector.tensor_copy(oe[0:33, icl0:icl0 + IC],
                                          op[0:33, 0:IC])
                    nc.vector.tensor_copy(oe[64:97, icl0:icl0 + IC],
                                          op[64:97, IC:2 * IC])

                if phases == 1:
                    if half == 0 and h == 0:
                        dbg1 = singles.tile([64, N // 2], BF16, name="dbg1")
                        nc.vector.tensor_copy(dbg1[:], oe[0:64, :])
                        nc.sync.dma_start(out=on_out[:, 0:N // 2], in_=dbg1[:])
                    continue

                # phase 2: transpose, normalize, transpose back
                outT = mpool.tile([128, 16 * 33], F32, tag="outT")
                for b4 in range(4):
                    tp = psum.tile([128, 4 * 98], F32, tag="small")
                    for s in range(4):
                        it = b4 * 4 + s
                        nc.tensor.matmul(
                            tp[:, s * 98:s * 98 + 97],
                            lhsT=oe[0:97, it * 128:(it + 1) * 128],
                            rhs=ident_f[0:97, 0:97],
                            is_transpose=True,
                        )
                    dst = outT[:, b4 * 132:(b4 + 1) * 132] \
                        .rearrange("p (s c) -> p s c", c=33)
                    tpv = tp[:].rearrange("p (s c) -> p s c", c=98)
                    nc.vector.tensor_copy(dst, tpv[:, :, 0:33])
                    nc.vector.tensor_add(dst, dst, tpv[:, :, 64:97])
                if phases in (2.05, 2.1):
                    continue
                outT_v = outT[:].rearrange("p (t c) -> p t c", c=33)
                recip = mpool.tile([128, 16], F32, tag="recip")
                nc.vector.reciprocal(recip[:], outT_v[:, :, 32:33])
                onT = mpool.tile([128, 512], BF16, tag="onT")
                for t in range(16):
                    nc.vector.tensor_scalar_mul(
                        onT[:, t * 32:(t + 1) * 32],
                        outT_v[:, t, 0:32],
                        recip[:, t:t + 1],
                    )
                if phases == 2.2:
                    continue
                for b4 in range(4):
                    tb = psum.tile([64, 512], BF16, tag="small")
                    for s in range(4):
                        it = b4 * 4 + s
                        nc.tensor.matmul(
                            tb[32 * h:32 * h + 32, s * 128:(s + 1) * 128],
                            lhsT=onT[:, it * 32:(it + 1) * 32],
                            rhs=ident_b[:, 0:128],
                            is_transpose=True,
                            tile_position=(0, 32 * h),
                        )
                    dst0 = half * (N // 2) + b4 * 512
                    nc.vector.tensor_copy(
                        on_sb[32 * h:32 * h + 32, dst0:dst0 + 512],
                        tb[32 * h:32 * h + 32, :],
                    )

            if phases < 3:
                continue
            # phase 3 (host): y = w_out @ on is done on the host after
            # download — on_sb is half the bytes of the y partial, and the
            # pair ReduceScatter disappears. Ship each i-half as it's done.
            nc.sync.dma_start(
                out=on_out[:, half * (N // 2):(half + 1) * (N // 2)],
                in_=on_sb[:, half * (N // 2):(half + 1) * (N // 2)],
            )


_CACHE = {}


def get_compiled(phases=3):
    key = ("nc", phases)
    if key not in _CACHE:
        nc = bacc.Bacc("TRN2", target_bir_lowering=False, debug=False,
                       num_devices=8)
        with tile.TileContext(nc) as tc:
            build_program(nc, tc, phases=phases)
        nc.compile()
        _CACHE[key] = nc
    return _CACHE[key]


def _bf(a):
    return np.ascontiguousarray(a.astype(ml_dtypes.bfloat16))


def prep_core_inputs(x, w_qkv):
    """Build the 8 per-core in_maps (host-side slicing/transposes).

    x is quantized to int8 with a per-(batch, channel) scale; the scales are
    folded into that batch's qkv weight copies, so the device works on
    (W diag(s)) @ xq with xq exact in bf16.
    """
    x = np.asarray(x, np.float32).reshape(B, C, N)
    w_qkv = np.asarray(w_qkv, np.float32)
    in_maps = []
    for core in range(8):
        b, pair = core // 2, core % 2
        ha, hb = 2 * pair, 2 * pair + 1
        s = np.abs(x[b]).max(axis=1) / 127.0          # [256] channel scales
        xq = np.clip(np.round(x[b] / s[:, None]), -127, 127).astype(np.int8)
        xh = np.ascontiguousarray(xq[:, pair * (N // 2):(pair + 1) * (N // 2)])
        ws = w_qkv * s[None, :]                       # fold scales

        def rep2(rows):  # [32, 256] weight rows -> [128, 128] replicated x2
            out = np.zeros((128, 128), np.float32)
            for cc in range(2):
                blk = rows[:, cc * 128:(cc + 1) * 128].T  # [128c, 32d]
                for r in range(2):
                    out[:, cc * 64 + r * 32: cc * 64 + (r + 1) * 32] = blk
            return _bf(out)

        wq = [rep2(ws[32 * hh:32 * hh + 32]) for hh in (ha, hb)]
        wk = [rep2(ws[128 + 32 * hh:128 + 32 * hh + 32]) for hh in (ha, hb)]
        wv = np.zeros((128, 194), np.float32)
        for cc in range(2):
            wv[:, cc * 97 + 0: cc * 97 + 32] = \
                ws[256 + 32 * ha:256 + 32 * ha + 32, cc * 128:(cc + 1) * 128].T
            wv[:, cc * 97 + 64: cc * 97 + 96] = \
                ws[256 + 32 * hb:256 + 32 * hb + 32, cc * 128:(cc + 1) * 128].T
        in_maps.append({
            "xh": xh, "wq0": wq[0], "wq1": wq[1], "wk0": wk[0], "wk1": wk[1],
            "wv": _bf(wv),
        })
    return in_maps


def _build_runner(nc, n_cores=8):
    """Cached equivalent of bass_utils.run_bass_kernel_spmd's execute path.

    run_bass_kernel_spmd -> run_bass_via_pjrt builds a fresh jax.jit closure
    on every call, so each invocation pays a full XLA retrace + compile
    round trip (~0.7s here) on top of the actual transfer + execute. This
    builds the identical shard_map program once and reuses it; every call
    still uploads inputs + donated zero outputs, executes the NEFF on all 8
    cores, and downloads the outputs.
    """
    import jax
    from concourse import bass2jax
    from jax.sharding import Mesh, PartitionSpec
    from jax.experimental.shard_map import shard_map

    bass2jax.install_neuronx_cc_hook()
    assert nc.dbg_addr is None and nc.partition_id_tensor is not None
    partition_name = nc.partition_id_tensor.name

    in_names, out_names, out_avals, zero_outs = [], [], [], []
    for alloc in nc.m.functions[0].allocations:
        if not isinstance(alloc, mybir.MemoryLocationSet):
            continue
        name = alloc.memorylocations[0].name
        if alloc.kind == "ExternalInput":
            if name != partition_name:
                in_names.append(name)
        elif alloc.kind == "ExternalOutput":
            shape = tuple(alloc.tensor_shape)
            dtype = mybir.dt.np(alloc.dtype)
            out_names.append(name)
            out_avals.append(jax.core.ShapedArray(shape, dtype))
            zero_outs.append(np.zeros((n_cores * shape[0], *shape[1:]), dtype))
    n_params = len(in_names)
    bind_in_names = tuple(in_names + out_names + [partition_name])
    donate = tuple(range(n_params, n_params + len(out_names)))

    def _body(*args):
        operands = list(args)
        operands.append(bass2jax.partition_id_tensor())
        outs = bass2jax._bass_exec_p.bind(
            *operands,
            out_avals=tuple(out_avals),
            in_names=bind_in_names,
            out_names=tuple(out_names),
            lowering_input_output_aliases=(),
            sim_require_finite=True,
            sim_require_nnan=True,
            nc=nc,
        )
        return tuple(outs)

    devices = jax.devices()[:n_cores]
    mesh = Mesh(np.asarray(devices), ("core",))
    in_specs = (PartitionSpec("core"),) * (n_params + len(out_names))
    out_specs = (PartitionSpec("core"),) * len(out_names)
    sharded = jax.jit(
        shard_map(_body, mesh=mesh, in_specs=in_specs, out_specs=out_specs,
                  check_rep=False),
        donate_argnums=donate, keep_unused=True,
    )

    # The kernel fully overwrites its ExternalOutput (one whole-tensor DMA),
    # so the donated output-scratch operand's *content* is irrelevant; only
    # its shape/sharding matter. First call donates host zeros; later calls
    # donate the previous call's device-resident output, avoiding an 8 MiB
    # host->device upload per invocation.
    scratch = list(zero_outs)

    def run(in_maps):
        concat_in = [
            np.concatenate([np.asarray(in_maps[c][nm]) for c in range(n_cores)],
                           axis=0)
            for nm in in_names
        ]
        out_arrs = sharded(*concat_in, *scratch)
        results = [
            {nm: np.asarray(out_arrs[i]).reshape(n_cores, *out_avals[i].shape)[c]
             for i, nm in enumerate(out_names)}
            for c in range(n_cores)
        ]
        scratch[:] = list(out_arrs)
        return bass_utils.BassKernelResults(
            results=results, instructions_and_trace=None,
            profile_json=None, exec_time_ns=None)

    return run


def run_cores(in_maps, trace=False):
    nc = get_compiled()
    if trace:
        return bass_utils.run_bass_kernel_spmd(
            nc, in_maps, core_ids=list(range(8)), trace=True)
    key = "runner"
    if key not in _CACHE:
        _CACHE[key] = _build_runner(nc)
    return _CACHE[key](in_maps)


def assemble_output(results, w_out, b_out):
    w_out = np.asarray(w_out, np.float32)
    b_out = np.asarray(b_out, np.float32)
    y = np.empty((B, C, N), np.float32)
    for b in range(B):
        # even core: heads 0,1 (rows 0:64 = w_out cols 0:64); odd: heads 2,3
        on = np.concatenate([
            results[2 * b]["on"].astype(np.float32),
            results[2 * b + 1]["on"].astype(np.float32),
        ], axis=0)  # [128, N] = normalized attention out, head-major rows
        y[b] = w_out @ on
    y += b_out[None, :, None]
    return y.reshape(B, C, H, W)


def kernel(x, w_qkv, w_out, b_out):
    in_maps = prep_core_inputs(x, w_qkv)
    res = run_cores(in_maps)
    return assemble_output(res.results, w_out, b_out)

